# revision 26
# baseline (speedup 1.0000x reference)
"""TRN2 Bass kernel for nn_Block_18227841204857 (EViT-style block with top-k token
merging). Data-parallel over batch: 8 cores x 16 samples.

Contract: kernel(**inputs) takes full unsharded inputs, returns full output
(128, 139, 768) float32.

Precision strategy: the top-k selection path (ln1, k-projection, cls-query q0,
cls scores, softmax-mean) stays true fp32; everything else off the selection
path runs f32r matmuls (1 cycle/row at free>=256 vs fp32's 4) or bf16 (MLP).
"""
import sys
sys.path.insert(0, "/opt/trn_rl_repo")

import math
import numpy as np

import concourse.bacc as bacc
import concourse.bass as bass
import concourse.mybir as mybir
from concourse.tile import TileContext
from concourse.masks import make_identity
from concourse.bass_utils import run_bass_kernel_spmd

P = 128
f32 = mybir.dt.float32
f32r = mybir.dt.float32r
bf16 = mybir.dt.bfloat16
i32 = mybir.dt.int32
AF = mybir.ActivationFunctionType
OP = mybir.AluOpType
AX = mybir.AxisListType

B_L = 16          # samples per core
N = 197           # tokens
C = 768           # channels
NH = 12           # heads
HD = 64           # head dim
L = 138           # kept tokens
M = 58            # pruned tokens
NO = 139          # output tokens (cls + kept)
H4 = 3072         # mlp hidden
EPS = 1e-5
NCHUNK = 8        # phase-A chunks (2 samples each)
T2 = 2 * N        # 394 tokens per chunk
QPAD = 256        # free size for f32r score matmuls (>=256 for 1 cycle/row)

LOG2E = float(np.float32(1.4426950408889634))
LN2 = float(np.float32(0.6931471805599453))
MAGIC = 12582912.0  # 1.5 * 2**23, round-to-nearest-int trick

KC = {}  # const tiles shared across build helpers


def _sample_tiles():
    # token tiles within one sample: (tile_idx, start, size)
    return [(0, 0, 128), (1, 128, 69)]


def _ceil_tiles(n):
    out = []
    s = 0
    while s < n:
        sz = min(P, n - s)
        out.append((s, sz))
        s += sz
    return out


def dve_exp(nc, sb, out_ap, in_ap, rows, cols,
            names=("exp_y", "exp_nf", "exp_t", "exp_p", "exp_ni")):
    """out = exp(in) elementwise, ~3e-7 rel accuracy, DVE+ACT only.

    exp(s) = 2^n * e^f,  n = round(s*log2e),  f = s - n*ln2  in [-0.347, 0.347]
    names lets callers alias the temporaries onto dead slots in their pool.
    """
    shp = [rows, cols]
    y = sb.tile(shp, f32, name=names[0])
    nf = sb.tile(shp, f32, name=names[1])
    t = sb.tile(shp, f32, name=names[2])
    p = sb.tile(shp, f32, name=names[3])
    u = y  # y is dead once nf is extracted; reuse as Horner ping-pong
    ni = sb.tile(shp, i32, name=names[4])

    nc.vector.tensor_scalar(out=y[:], in0=in_ap, scalar1=LOG2E, scalar2=MAGIC,
                            op0=OP.mult, op1=OP.add)
    nc.vector.tensor_scalar(out=nf[:], in0=y[:], scalar1=MAGIC, scalar2=None,
                            op0=OP.subtract)
    # t = s - nf*ln2   (scalar_tensor_tensor: (in0 op0 scalar) op1 in1)
    nc.vector.scalar_tensor_tensor(out=t[:], in0=nf[:], scalar=-LN2, in1=in_ap,
                                   op0=OP.mult, op1=OP.add)
    # Horner for e^t, t in [-0.35, 0.35]: coeffs 1/k!, k=8..0
    coef = [1.0 / math.factorial(k) for k in range(9)]
    nc.vector.tensor_scalar(out=p[:], in0=t[:], scalar1=coef[8], scalar2=coef[7],
                            op0=OP.mult, op1=OP.add)
    for k in range(6, -1, -1):
        nc.vector.tensor_tensor(out=u[:], in0=p[:], in1=t[:], op=OP.mult)
        nc.vector.tensor_scalar(out=p[:], in0=u[:], scalar1=coef[k], scalar2=None,
                                op0=OP.add)
    # 2^n via bit tricks: (n + 127) << 23, bitcast to f32
    nc.vector.tensor_copy(ni[:], nf[:])
    nc.vector.tensor_scalar(out=ni[:], in0=ni[:], scalar1=127, scalar2=None, op0=OP.add)
    nc.vector.tensor_scalar(out=ni[:], in0=ni[:], scalar1=23, scalar2=None,
                            op0=OP.arith_shift_left)
    nc.vector.tensor_tensor(out=out_ap, in0=p[:], in1=ni[:].bitcast(f32), op=OP.mult)


def newton_recip(nc, sb, out_ap, in_ap, shp, tag):
    """out = 1/in with one Newton refinement on DVE reciprocal."""
    r0 = sb.tile(shp, f32, name=f"nr_r0_{tag}")
    tt = sb.tile(shp, f32, name=f"nr_t_{tag}")
    nc.vector.reciprocal(r0[:], in_ap)
    nc.vector.tensor_tensor(out=tt[:], in0=r0[:], in1=in_ap, op=OP.mult)
    nc.vector.tensor_scalar(out=tt[:], in0=tt[:], scalar1=-1.0, scalar2=2.0,
                            op0=OP.mult, op1=OP.add)
    nc.vector.tensor_tensor(out=out_ap, in0=r0[:], in1=tt[:], op=OP.mult)


def dve_rsqrt(nc, sb, out_ap, in_ap, shp, tag):
    """out = 1/sqrt(in) on DVE only (quake seed + 3 Newton iters)."""
    v = sb.tile(shp, f32, name=f"rq_v_{tag}")
    yb = sb.tile(shp, i32, name=f"rq_yb_{tag}")
    t1 = sb.tile(shp, f32, name=f"rq_t1_{tag}")
    nc.vector.tensor_copy(v[:], in_ap)
    nc.vector.tensor_scalar(out=yb[:], in0=v[:].bitcast(i32), scalar1=1,
                            scalar2=None, op0=OP.arith_shift_right)
    nc.vector.tensor_scalar(out=yb[:], in0=yb[:], scalar1=-1,
                            scalar2=0x5F3759DF, op0=OP.mult, op1=OP.add)
    y = yb[:].bitcast(f32)
    for _ in range(3):
        nc.vector.tensor_tensor(out=t1[:], in0=v[:], in1=y, op=OP.mult)
        nc.vector.tensor_tensor(out=t1[:], in0=t1[:], in1=y, op=OP.mult)
        nc.vector.tensor_scalar(out=t1[:], in0=t1[:], scalar1=-0.5, scalar2=1.5,
                                op0=OP.mult, op1=OP.add)
        nc.vector.tensor_tensor(out=out_ap, in0=y, in1=t1[:], op=OP.mult)
        y = out_ap


def layernorm_tile(nc, sb, ps, out_ap, x_ap, rows, newton, tag):
    """LayerNorm over free dim (768), w=1 b=0. out/x: [rows, C]."""
    shp = [rows, 1]
    rs = sb.tile(shp, f32, name=f"ln_rs_{tag}")
    nm = sb.tile(shp, f32, name=f"ln_nm_{tag}")
    xc = sb.tile([rows, C], f32, name=f"ln_xc_{tag}")
    sq = sb.tile([rows, C], f32, name=f"ln_sq_{tag}")
    vs = sb.tile(shp, f32, name=f"ln_vs_{tag}")
    sd = sb.tile(shp, f32, name=f"ln_sd_{tag}")
    nc.vector.tensor_reduce(out=rs[:], in_=x_ap, axis=AX.X, op=OP.add)
    nc.vector.tensor_scalar(out=nm[:], in0=rs[:], scalar1=-1.0 / C, scalar2=None,
                            op0=OP.mult)
    nc.vector.tensor_scalar(out=xc[:], in0=x_ap, scalar1=nm[:, 0:1], scalar2=None,
                            op0=OP.add)
    nc.vector.tensor_tensor(out=sq[:], in0=xc[:], in1=xc[:], op=OP.mult)
    nc.vector.tensor_reduce(out=vs[:], in_=sq[:], axis=AX.X, op=OP.add)
    nc.scalar.activation(sd[:], vs[:], AF.Sqrt, scale=1.0 / C,
                         bias=KC["eps"][0:rows, 0:1])
    inv = sb.tile(shp, f32, name=f"ln_inv_{tag}")
    if newton:
        newton_recip(nc, sb, inv[:], sd[:], shp, f"ln_{tag}")
    else:
        nc.vector.reciprocal(inv[:], sd[:])
    nc.vector.tensor_scalar(out=out_ap, in0=xc[:], scalar1=inv[:, 0:1], scalar2=None,
                            op0=OP.mult)


def build_nc():
    nc = bacc.Bacc("TRN2", target_bir_lowering=False, debug=False, num_swdge_queues=1)

    x_in = nc.declare_dram_parameter("x", [B_L, N, C], f32, isOutput=False)
    qkv_w = nc.declare_dram_parameter("qkv_w", [C, 3 * C], f32, isOutput=False)
    proj_w = nc.declare_dram_parameter("proj_w", [C, C], f32, isOutput=False)
    fc1_w = nc.declare_dram_parameter("fc1_w", [C, H4], bf16, isOutput=False)
    fc2_w = nc.declare_dram_parameter("fc2_w", [H4, C], bf16, isOutput=False)
    out_ext = nc.declare_dram_parameter("out", [B_L, NO, C], f32, isOutput=True)

    out_flat = out_ext.ap().rearrange("b n c -> (b n) c")

    with TileContext(nc) as tc:
        _build_body(nc, tc, x_in, qkv_w, proj_w, fc1_w, fc2_w, out_flat)
    nc.finalize()
    return nc


def _build_body(nc, tc, x_in, qkv_w, proj_w, fc1_w, fc2_w, out_flat):
    from contextlib import ExitStack

    ctx = ExitStack()
    with ctx:
        # ---------- constants ----------
        pc = ctx.enter_context(tc.tile_pool(name="const", bufs=1))
        pdram = ctx.enter_context(tc.tile_pool(name="dram", bufs=1, space="DRAM"))

        ident = pc.tile([P, P], f32)
        make_identity(nc, ident[:])
        onesP = pc.tile([P, 1], f32)
        nc.vector.memset(onesP[:], 1.0)
        onesRow = pc.tile([1, P], f32)
        nc.vector.memset(onesRow[:], 1.0)
        epsb = pc.tile([P, 1], f32)
        nc.vector.memset(epsb[:], EPS)
        zerob = pc.tile([P, 1], f32)
        nc.vector.memset(zerob[:], 0.0)
        KC["eps"] = epsb
        KC["zero"] = zerob

        iota_i = pc.tile([P, 1], i32)
        nc.gpsimd.iota(iota_i[:], pattern=[[1, 1]], base=0, channel_multiplier=1)
        iotaPf = pc.tile([P, 1], f32)
        nc.vector.tensor_copy(iotaPf[:], iota_i[:])
        iotaPf128 = pc.tile([P, 1], f32)
        nc.vector.tensor_scalar(out=iotaPf128[:], in0=iotaPf[:], scalar1=128.0,
                                scalar2=None, op0=OP.add)

        iota_f_i = pc.tile([P, 196], i32)
        nc.gpsimd.iota(iota_f_i[:], pattern=[[1, 196]], base=0, channel_multiplier=0)
        IotaF = pc.tile([P, 196], f32)
        nc.vector.tensor_copy(IotaF[:], iota_f_i[:])
        IotaLmB = pc.tile([P, L], f32)
        nc.vector.tensor_scalar(out=IotaLmB[:], in0=IotaF[:, 0:L], scalar1=-65536.0,
                                scalar2=None, op0=OP.add)

        LT0 = pc.tile([P, 196], f32)
        nc.vector.tensor_scalar(out=LT0[:], in0=IotaF[:], scalar1=iotaPf[:, 0:1],
                                scalar2=None, op0=OP.is_lt)
        LT1 = pc.tile([P, 196], f32)
        nc.vector.tensor_scalar(out=LT1[:], in0=IotaF[:], scalar1=iotaPf128[:, 0:1],
                                scalar2=None, op0=OP.is_lt)
        TRI0 = pc.tile([P, 196], f32)
        nc.vector.tensor_scalar(out=TRI0[:], in0=IotaF[:], scalar1=iotaPf[:, 0:1],
                                scalar2=None, op0=OP.is_ge)
        TRI1 = pc.tile([P, 196], f32)
        nc.vector.tensor_scalar(out=TRI1[:], in0=IotaF[:], scalar1=iotaPf128[:, 0:1],
                                scalar2=None, op0=OP.is_ge)

        # ---------- DRAM scratch ----------
        xattn_d = pdram.tile([B_L * N, C], f32)
        xsel_d = pdram.tile([B_L * NO, C], f32)
        clsD = pdram.tile([B_L, 196], f32)

        # ================= PHASE A + interleaved selection =================
        with ExitStack() as actx:
            pw = actx.enter_context(tc.tile_pool(name="aw", bufs=1))
            pp = actx.enter_context(tc.tile_pool(name="aps", bufs=6, space="PSUM"))

            # resident weights: all f32r; the fp32 selection path uses the
            # precomputed u-vectors (u_h = Wk_h @ q0_h) instead of fp32 k
            qkvKr = pw.tile([P, 6, C], f32r)
            qkvQr = pw.tile([P, 6, C], f32r)
            qkvVr = pw.tile([P, 6, C], f32r)
            projr = pw.tile([P, 6, C], f32r)
            u_all = pw.tile([P, 6, B_L * NH], f32)

            with ExitStack() as sctx:
                pstage = sctx.enter_context(tc.tile_pool(name="stg", bufs=1))
                # separate staging tiles so the three weight loads + casts all
                # run in parallel (across DMA and the three copy engines)
                wq = pstage.tile([P, 6, C], f32, name="wq")
                wv = pstage.tile([P, 6, C], f32, name="wv")
                wp = pstage.tile([P, 6, C], f32, name="wp")
                wk = pstage.tile([P, 6, C], f32, name="wk")
                nc.sync.dma_start(
                    out=wq[:],
                    in_=qkv_w.ap()[:, 0:C].rearrange("(kt p) o -> p kt o", p=P))
                nc.sync.dma_start(
                    out=wv[:],
                    in_=qkv_w.ap()[:, 2 * C:3 * C].rearrange("(kt p) o -> p kt o", p=P))
                nc.sync.dma_start(
                    out=wp[:],
                    in_=proj_w.ap().rearrange("(kt p) o -> p kt o", p=P))
                nc.sync.dma_start(
                    out=wk[:],
                    in_=qkv_w.ap()[:, C:2 * C].rearrange("(kt p) o -> p kt o", p=P))
                nc.scalar.copy(qkvVr[:], wv[:])
                nc.vector.tensor_copy(projr[:], wp[:])
                nc.scalar.copy(qkvKr[:], wk[:])
                # Wk^T (fp32, exact transposes) for the u-vector pre-pass
                wkT = pstage.tile([P, 6, C], f32, name="wkT")
                for bt in range(6):
                    ptw = pp.tile([P, P], f32, space="PSUM", name="ps")
                    for at in range(6):
                        nc.tensor.transpose(
                            out=ptw[:, 0:P],
                            in_=wk[:, at, bt * P:(bt + 1) * P],
                            identity=ident[:])
                        nc.vector.tensor_copy(wkT[:, bt, at * P:(at + 1) * P],
                                              ptw[:, 0:P])
                # --- q0 pre-pass: fp32 cls-query for all 16 samples ---
                xcls = pstage.tile([B_L, C], f32, name="xcls")
                nc.sync.dma_start(out=xcls[:], in_=x_in.ap()[:, 0, :])
                xn0 = pstage.tile([B_L, C], f32, name="xn0")
                layernorm_tile(nc, pstage, pp, xn0[:], xcls[:], B_L, True, "q0")
                xn0T = pstage.tile([P, 6, B_L], f32, name="xn0T")
                for ci in range(6):
                    ptq = pp.tile([P, B_L], f32, space="PSUM", name="ps")
                    nc.tensor.transpose(out=ptq[:, 0:B_L],
                                        in_=xn0[0:B_L, ci * P:(ci + 1) * P],
                                        identity=ident[0:B_L, 0:B_L])
                    nc.vector.tensor_copy(xn0T[:, ci, :], ptq[:])
                q0_all = pstage.tile([B_L, C], f32, name="q0_all")
                for (n0, nsz) in ((0, 512), (512, 256)):
                    psq = pp.tile([B_L, 512], f32, space="PSUM", name="ps")
                    for ki in range(6):
                        nc.tensor.matmul(psq[0:B_L, 0:nsz],
                                         lhsT=xn0T[:, ki, :],
                                         rhs=wq[:, ki, n0:n0 + nsz],
                                         start=(ki == 0), stop=(ki == 5))
                    nc.vector.tensor_copy(q0_all[:, n0:n0 + nsz], psq[0:B_L, 0:nsz])
                q0T_all = pstage.tile([P, 6, B_L], f32, name="q0T_all")
                for ci in range(6):
                    ptq = pp.tile([P, B_L], f32, space="PSUM", name="ps")
                    nc.tensor.transpose(out=ptq[:, 0:B_L],
                                        in_=q0_all[0:B_L, ci * P:(ci + 1) * P],
                                        identity=ident[0:B_L, 0:B_L])
                    nc.vector.tensor_copy(q0T_all[:, ci, :], ptq[:])
                nc.vector.tensor_copy(qkvQr[:], wq[:])
                # u[c, (samp, h)] = sum_d Wk[c, h*64+d] * q0[samp, h*64+d]
                # (fp32; replaces the fp32 k-projection in the score path)
                for h in range(NH):
                    bt, off = h // 2, (h % 2) * HD
                    for ft in range(6):
                        psu = pp.tile([P, B_L], f32, space="PSUM", name="ps")
                        nc.tensor.matmul(
                            psu[:],
                            lhsT=wkT[off:off + HD, bt, ft * P:(ft + 1) * P],
                            rhs=q0T_all[off:off + HD, bt, :],
                            start=True, stop=True)
                        dst = u_all[:, ft, :].rearrange(
                            "p (s h) -> p s h", h=NH)[:, :, h]
                        nc.vector.tensor_copy(dst, psu[:])

            # main working pools enter AFTER the staging scope exits so its
            # SBUF is reclaimed (stack LIFO)
            pa = actx.enter_context(tc.tile_pool(name="aa", bufs=1))
            pa2 = actx.enter_context(tc.tile_pool(name="aa2", bufs=1))
            pb2 = actx.enter_context(tc.tile_pool(name="b2i", bufs=1))

            # software pipeline: emit chunk ch's attention + its cls-score
            # extraction, then the PREVIOUS chunk's selection/merge work so its
            # vector-engine chain overlaps chunk ch's PE-bound matmuls.
            for ch in range(NCHUNK):
                xnT = _phase_a_chunk(nc, tc, pa, pa2, pp, ch, x_in, qkvKr,
                                     qkvQr, qkvVr, projr, ident, onesP,
                                     xattn_d, xsel_d)
                if ch > 0:
                    _phase_b_rest(nc, tc, pb2, pp, ch - 1, xattn_d, xsel_d,
                                  clsD, LT0, LT1, TRI0, TRI1, IotaF, IotaLmB,
                                  onesP, onesRow, iotaPf, iotaPf128, ident)
                _phase_b_score(nc, pb2, pp, ch, u_all, xnT)
            _phase_b_rest(nc, tc, pb2, pp, NCHUNK - 1, xattn_d, xsel_d,
                          clsD, LT0, LT1, TRI0, TRI1, IotaF, IotaLmB,
                          onesP, onesRow, iotaPf, iotaPf128, ident)

        # ================= PHASE C: MLP =================
        with ExitStack() as cctx:
            pw = cctx.enter_context(tc.tile_pool(name="cw", bufs=1))
            pcs = cctx.enter_context(tc.tile_pool(name="cc", bufs=2))
            pc1 = cctx.enter_context(tc.tile_pool(name="cc1", bufs=1))
            pp = cctx.enter_context(tc.tile_pool(name="cps", bufs=6, space="PSUM"))

            fc2_r = pw.tile([P, 24, C], bf16)
            w2view = fc2_w.ap().rearrange("(kt p) o -> p kt o", p=P)
            nc.sync.dma_start(out=fc2_r[:], in_=w2view[:])
            fc1_r = pw.tile([P, 6, H4], bf16)
            nc.sync.dma_start(out=fc1_r[:],
                              in_=fc1_w.ap().rearrange("(kt p) o -> p kt o", p=P))

            TOK = B_L * NO  # 2224
            TCK = 512
            nch = (TOK + TCK - 1) // TCK
            for ci in range(nch):
                t0 = ci * TCK
                tsz = min(TCK, TOK - t0)
                _phase_c_chunk(nc, tc, pcs, pp, t0, tsz, xsel_d, fc1_r,
                               fc2_r, ident, out_flat, pc1)


def _phase_b_score(nc, pb, pp, ch, u_all, xnT):
    """Raw cls scores for samples 2ch, 2ch+1: s[h, j] = xn_j . u_h (fp32),
    mathematically q0_h . k_{j,h} with the Wk contraction pre-folded into u."""
    s_all = pb.tile([NH, 2, N], f32, name="s_all")
    for s2 in range(2):
        samp = 2 * ch + s2
        psc = pp.tile([NH, N], f32, space="PSUM", name="ps")
        for ki in range(6):
            nc.tensor.matmul(psc[:],
                             lhsT=u_all[:, ki, samp * NH:(samp + 1) * NH],
                             rhs=xnT[:, ki, s2 * N:(s2 + 1) * N],
                             start=(ki == 0), stop=(ki == 5))
        nc.vector.tensor_scalar(out=s_all[:, s2, :], in0=psc[:],
                                scalar1=0.125, scalar2=None, op0=OP.mult)


def _phase_b_rest(nc, tc, pb, pp, ch, xattn_d, xsel_d, clsD,
                  LT0, LT1, TRI0, TRI1, IotaF, IotaLmB, onesP, onesRow,
                  iotaPf, iotaPf128, ident):
    """Softmax + topk rank + token merge for chunk ch, emitted one chunk late
    so its vector-engine chain overlaps the next chunk's PE-bound matmuls."""
    s_all = pb.tile([NH, 2, N], f32, name="s_all")
    # max-subtracted exp + softmax + head mean
    smax = pb.tile([NH, 2], f32, name="smax")
    nc.vector.tensor_reduce(out=smax[:], in_=s_all[:], axis=AX.X, op=OP.max)
    sbc = bass.AP(tensor=smax[:].tensor, offset=smax[:].offset,
                  ap=[[smax[:].ap[0][0], NH], [1, 2], [0, N]])
    nc.vector.tensor_tensor(out=s_all[:], in0=s_all[:], in1=sbc, op=OP.subtract)
    nc.vector.tensor_scalar(out=s_all[:], in0=s_all[:], scalar1=-80.0,
                            scalar2=None, op0=OP.max)
    e_all = pb.tile([NH, 2, N], f32, name="e_all")
    # temporaries alias phase-b2 slots that are dead at this point in the chunk
    dve_exp(nc, pb, e_all[:].rearrange("h s n -> h (s n)"),
            s_all[:].rearrange("h s n -> h (s n)"), NH, 2 * N,
            names=("gath0", "gath1", "gath2", "x_nc", "xntT"))
    den = pb.tile([NH, 2], f32, name="den")
    nc.vector.tensor_reduce(out=den[:], in_=e_all[:], axis=AX.X, op=OP.add)
    rden = smax  # smax is dead after the subtract above
    newton_recip(nc, pb, rden[:], den[:], [NH, 2], "den")
    a_all = e_all  # normalize in place; raw exps are dead after this
    rbc = bass.AP(tensor=rden[:].tensor, offset=rden[:].offset,
                  ap=[[rden[:].ap[0][0], NH], [1, 2], [0, N]])
    nc.vector.tensor_tensor(out=a_all[:], in0=e_all[:], in1=rbc, op=OP.mult)
    cls_all = pb.tile([1, 2, 196], f32, name="cls_all")
    for s2 in range(2):
        pcm = pp.tile([1, 196], f32, space="PSUM", name="ps")
        nc.tensor.matmul(pcm[:], lhsT=onesP[0:NH, 0:1],
                         rhs=a_all[:, s2, 1:N], start=True, stop=True)
        nc.vector.tensor_scalar(out=cls_all[0:1, s2, :], in0=pcm[:],
                                scalar1=1.0 / 12.0, scalar2=None, op0=OP.mult)

    # per-key cls values, [i(2 tiles), s2]
    clsPT = pb.tile([P, 2, 2], f32, name="clsPT")
    for s2 in range(2):
        ptt = pp.tile([P, 1], f32, space="PSUM", name="ps")
        nc.tensor.transpose(out=ptt[:, 0:1], in_=cls_all[0:1, s2, 0:P],
                            identity=ident[0:1, 0:1])
        nc.vector.tensor_copy(clsPT[:, 0, s2:s2 + 1], ptt[:, 0:1])
        ptt2 = pp.tile([P, 1], f32, space="PSUM", name="ps")
        nc.tensor.transpose(out=ptt2[0:68, 0:1], in_=cls_all[0:1, s2, P:196],
                            identity=ident[0:1, 0:1])
        nc.vector.tensor_copy(clsPT[0:68, 1, s2:s2 + 1], ptt2[0:68, 0:1])

    # ranks, one sample at a time (VF: cls row broadcast to all partitions via
    # a rank-1 PE matmul into PSUM — avoids a DRAM round-trip on the critical
    # chain; the compare ops read the psum directly)
    rank_all = pb.tile([P, 2, 2], f32, name="rank_all")
    pruned_all = pb.tile([P, 2, 2], f32, name="pruned_all")
    for s2 in range(2):
        VFp = pp.tile([P, 196], f32, space="PSUM", name="ps")
        nc.tensor.matmul(VFp[:], lhsT=onesRow[0:1, 0:P],
                         rhs=cls_all[0:1, s2, :], start=True, stop=True)
        VF = pb.tile([P, 196], f32, name="VF")
        nc.scalar.copy(VF[:], VFp[:])
        for t, lt in ((0, LT0), (1, LT1)):
            src = clsPT[:, t, s2:s2 + 1]
            vp = bass.AP(tensor=src.tensor, offset=src.offset,
                         ap=[src.ap[0], [0, 196]])
            gt = pb.tile([P, 196], f32, name="rk_gt")
            eq = pb.tile([P, 196], f32, name="rk_eq")
            nc.vector.tensor_tensor(out=gt[:], in0=VF[:], in1=vp, op=OP.is_gt)
            nc.vector.tensor_tensor(out=eq[:], in0=VF[:], in1=vp, op=OP.is_equal)
            nc.vector.tensor_tensor(out=eq[:], in0=eq[:], in1=lt[:], op=OP.mult)
            nc.vector.tensor_tensor(out=gt[:], in0=gt[:], in1=eq[:], op=OP.add)
            nc.vector.tensor_reduce(out=rank_all[:, t, s2:s2 + 1], in_=gt[:],
                                    axis=AX.X, op=OP.add)
            nc.vector.tensor_scalar(out=pruned_all[:, t, s2:s2 + 1],
                                    in0=rank_all[:, t, s2:s2 + 1],
                                    scalar1=137.5, scalar2=None, op0=OP.is_gt)
    # inclusive cumsum of pruned -> pos
    posP = pb.tile([P, 2, 2], f32, name="posP")
    pp0 = pp.tile([P, 2], f32, space="PSUM", name="ps")
    nc.tensor.matmul(pp0[:], lhsT=TRI0[:, 0:P], rhs=pruned_all[:, 0, :],
                     start=True, stop=False)
    nc.tensor.matmul(pp0[:], lhsT=TRI1[0:68, 0:P], rhs=pruned_all[0:68, 1, :],
                     start=False, stop=True)
    nc.vector.tensor_copy(posP[:, 0, :], pp0[:])
    pp1 = pp.tile([P, 2], f32, space="PSUM", name="ps")
    nc.tensor.matmul(pp1[0:68, :], lhsT=TRI0[:, P:196],
                     rhs=pruned_all[:, 0, :], start=True, stop=False)
    nc.tensor.matmul(pp1[0:68, :], lhsT=TRI1[0:68, P:196],
                     rhs=pruned_all[0:68, 1, :], start=False, stop=True)
    nc.vector.tensor_copy(posP[0:68, 1, :], pp1[0:68, :])

    for s2 in range(2):
        _phase_b2_sample(nc, tc, pb, pp, 2 * ch + s2, s2, xattn_d, xsel_d,
                         rank_all, pruned_all, posP, clsPT, IotaF, IotaLmB,
                         onesP, onesRow, iotaPf, iotaPf128, ident)


def _phase_a_chunk(nc, tc, pa, pa2, pp, ch, x_in, qkvKr, qkvQr, qkvVr, projr,
                   ident, onesP, xattn_d, xsel_d):
    st = _sample_tiles()
    x_sb = pa2.tile([P, 2, 2, C], f32, name="x_sb")
    xn_sb = pa.tile([P, 2, 2, C], f32, name="xn_sb")
    for s2 in range(2):
        samp = 2 * ch + s2
        for (mt, m0, msz) in st:
            nc.sync.dma_start(out=x_sb[0:msz, s2, mt, :],
                              in_=x_in.ap()[samp, m0:m0 + msz, :])
            layernorm_tile(nc, pa, pp, xn_sb[0:msz, s2, mt, :],
                           x_sb[0:msz, s2, mt, :], msz, True, "a")

    # transpose ln1 out -> feature-major [C, T2]: fp32 + f32r twin copies
    xnT = pa.tile([P, 6, T2], f32, name="xnT")
    xnTr = pa.tile([P, 6, T2], f32r, name="xnTr")
    for ci in range(6):
        ptr = pp.tile([P, T2], f32, space="PSUM", name="ps")
        for s2 in range(2):
            for (mt, m0, msz) in st:
                nc.tensor.transpose(
                    out=ptr[:, s2 * N + m0: s2 * N + m0 + msz],
                    in_=xn_sb[0:msz, s2, mt, ci * P:(ci + 1) * P],
                    identity=ident[0:msz, 0:msz])
        nc.vector.tensor_copy(xnT[:, ci, :], ptr[:])
        nc.scalar.copy(xnTr[:, ci, :], ptr[:])

    # q feature-major f32r
    qTr = pa.tile([P, 6, T2], f32r, name="qTr")
    for oi in range(6):
        pq = pp.tile([P, T2], f32, space="PSUM", name="ps")
        for ki in range(6):
            nc.tensor.matmul(
                pq[:],
                lhsT=qkvQr[:, ki, oi * P:(oi + 1) * P],
                rhs=xnTr[:, ki, :], start=(ki == 0), stop=(ki == 5))
        nc.vector.tensor_copy(qTr[:, oi, :], pq[:])

    # v token-major with leading ones column per head: [tok, (12, 65)]
    v_blk = pa.tile([P, 2, 2, NH, 65], f32, name="v_blk")
    nc.vector.memset(v_blk[:, :, :, :, 0:1], 1.0)
    for s2 in range(2):
        for (mt, m0, msz) in st:
            for nc_i, (n0, nsz) in enumerate(((0, 512), (512, 256))):
                pv = pp.tile([P, 512], f32, space="PSUM", name="ps")
                for ki in range(6):
                    nc.tensor.matmul(
                        pv[0:msz, 0:nsz],
                        lhsT=xnTr[:, ki, s2 * N + m0: s2 * N + m0 + msz],
                        rhs=qkvVr[:, ki, n0:n0 + nsz],
                        start=(ki == 0), stop=(ki == 5))
                h0 = n0 // HD
                nhh = nsz // HD
                nc.vector.tensor_copy(v_blk[0:msz, s2, mt, h0:h0 + nhh, 1:65],
                                      pv[0:msz, 0:nsz].rearrange(
                                          "p (h d) -> p h d", d=HD))

    # k feature-major f32r (the fp32 selection path no longer needs k — the
    # scores use the u-vectors against fp32 xnT). kTr reuses xnTr's slot
    # (the v section above is xnTr's last reader).
    kTr = pa.tile([P, 6, T2], f32r, name="xnTr")
    for oi in range(6):
        pq = pp.tile([P, T2], f32, space="PSUM", name="ps")
        for ki in range(6):
            nc.tensor.matmul(
                pq[:],
                lhsT=qkvKr[:, ki, oi * P:(oi + 1) * P],
                rhs=xnTr[:, ki, :], start=(ki == 0), stop=(ki == 5))
        nc.scalar.copy(kTr[:, oi, :], pq[:])

    # attention per head: scoresT (f32r, padded free) -> exp -> AV (+denom via
    # ones col) -> scale
    eT = pa.tile([P, 2, 2, N], f32, name="eT_xa")
    attn_out = pa.tile([P, 2, 2, C], f32, name="xn_sb")
    rr = pa.tile([P, 2, 2, NH], f32, name="rr")
    for h in range(NH):
        ci, off = h // 2, (h % 2) * HD
        for s2 in range(2):
            # 256-wide window keeps the f32r matmul at 1 cycle/row; for s2=1
            # the window is right-aligned (cols 138:394), so query j lands at
            # psum column 59+j instead of j.
            w0 = 0 if s2 == 0 else T2 - QPAD
            qo = s2 * N - w0
            for (nkt, k0, ksz) in st:
                psc = pp.tile([P, QPAD], f32, space="PSUM", name="ps")
                nc.tensor.matmul(
                    psc[0:ksz, :],
                    lhsT=kTr[off:off + HD, ci, s2 * N + k0: s2 * N + k0 + ksz],
                    rhs=qTr[off:off + HD, ci, w0: w0 + QPAD],
                    start=True, stop=True)
                nc.scalar.activation(eT[0:ksz, s2, nkt, :],
                                     psc[0:ksz, qo: qo + N],
                                     AF.Exp, scale=0.125,
                                     bias=KC["zero"][0:ksz, 0:1])
        for s2 in range(2):
            for (qt, q0, qsz) in st:
                po = pp.tile([P, 65], f32, space="PSUM", name="ps")
                for (nkt, k0, ksz) in st:
                    nc.tensor.matmul(
                        po[0:qsz, :],
                        lhsT=eT[0:ksz, s2, nkt, q0:q0 + qsz],
                        rhs=v_blk[0:ksz, s2, nkt, h, :],
                        start=(nkt == 0), stop=(nkt == 1))
                nc.vector.reciprocal(rr[0:qsz, s2, qt, h:h + 1], po[0:qsz, 0:1])
                nc.scalar.mul(attn_out[0:qsz, s2, qt, h * HD:(h + 1) * HD],
                              po[0:qsz, 1:65],
                              rr[0:qsz, s2, qt, h:h + 1])

    # transpose attn_out -> feature-major f32r (reuses qTr's slot: the score
    # matmuls above are qTr's last readers)
    aoTr = pa.tile([P, 6, T2], f32r, name="qTr")
    for ci in range(6):
        ptr = pp.tile([P, T2], f32, space="PSUM", name="ps")
        for s2 in range(2):
            for (mt, m0, msz) in st:
                nc.tensor.transpose(
                    out=ptr[:, s2 * N + m0: s2 * N + m0 + msz],
                    in_=attn_out[0:msz, s2, mt, ci * P:(ci + 1) * P],
                    identity=ident[0:msz, 0:msz])
        nc.scalar.copy(aoTr[:, ci, :], ptr[:])

    # proj (f32r) + residual -> xattn
    xa_sb = pa.tile([P, 2, 2, C], f32, name="eT_xa")
    for s2 in range(2):
        samp = 2 * ch + s2
        for (mt, m0, msz) in st:
            for (n0, nsz) in ((0, 512), (512, 256)):
                pj = pp.tile([P, 512], f32, space="PSUM", name="ps")
                for ki in range(6):
                    nc.tensor.matmul(
                        pj[0:msz, 0:nsz],
                        lhsT=aoTr[:, ki, s2 * N + m0: s2 * N + m0 + msz],
                        rhs=projr[:, ki, n0:n0 + nsz],
                        start=(ki == 0), stop=(ki == 5))
                nc.vector.tensor_tensor(out=xa_sb[0:msz, s2, mt, n0:n0 + nsz],
                                        in0=x_sb[0:msz, s2, mt, n0:n0 + nsz],
                                        in1=pj[0:msz, 0:nsz], op=OP.add)
            nc.sync.dma_start(out=xattn_d[samp * N + m0: samp * N + m0 + msz, :],
                              in_=xa_sb[0:msz, s2, mt, :])
        # cls row into xsel
        nc.sync.dma_start(out=xsel_d[samp * NO: samp * NO + 1, :],
                          in_=xa_sb[0:1, s2, 0, :])
    return xnT


def _phase_b2_sample(nc, tc, pb, pp, samp, s2, xattn_d, xsel_d, rank_all,
                     pruned_all, posP, clsPT, IotaF, IotaLmB, onesP, onesRow,
                     iotaPf, iotaPf128, ident):
    s = s2
    # one-hot selection matrices PT_cat [i, 196]: cols 0:138 keep, 138:196 compl
    # (f32r: exact 0/1 values, feeds the f32r gather matmuls)
    PTf = pb.tile([P, 2, 196], f32, name="PTf")
    for t, tsz in ((0, P), (1, 68)):
        nc.vector.tensor_scalar(out=PTf[0:tsz, t, 0:L], in0=IotaF[0:tsz, 0:L],
                                scalar1=rank_all[0:tsz, t, s:s + 1],
                                scalar2=None, op0=OP.is_equal)
        nc.vector.tensor_scalar(out=PTf[0:tsz, t, L:196], in0=IotaF[0:tsz, 1:59],
                                scalar1=posP[0:tsz, t, s:s + 1],
                                scalar2=None, op0=OP.is_equal)
        nc.vector.tensor_scalar(out=PTf[0:tsz, t, L:196], in0=PTf[0:tsz, t, L:196],
                                scalar1=pruned_all[0:tsz, t, s:s + 1],
                                scalar2=None, op0=OP.mult)

    mslices = ((0, P), (P, 10), (L, M))   # keep0, keep1, compl
    # gather cls_attn values (fp32: tiny)
    attnG = []
    for (ms0, mssz) in mslices:
        pg = pp.tile([P, 1], f32, space="PSUM", name="ps")
        for t, tsz in ((0, P), (1, 68)):
            nc.tensor.matmul(pg[0:mssz, :], lhsT=PTf[0:tsz, t, ms0:ms0 + mssz],
                             rhs=clsPT[0:tsz, t, s:s + 1],
                             start=(t == 0), stop=(t == 1))
        ga = pb.tile([P, 1], f32, name=f"attn_g{ms0}")
        nc.vector.tensor_copy(ga[0:mssz, :], pg[0:mssz, :])
        attnG.append(ga)

    # load non-cls rows and gather selected rows via exact one-hot matmuls (f32r)
    x_nc = pb.tile([P, 2, C], f32, name="x_nc")
    nc.sync.dma_start(out=x_nc[:, 0, :],
                      in_=xattn_d[samp * N + 1: samp * N + 129, :])
    nc.sync.dma_start(out=x_nc[0:68, 1, :],
                      in_=xattn_d[samp * N + 129: samp * N + 197, :])
    gath = []
    for gi, (ms0, mssz) in enumerate(mslices):
        gt = pb.tile([P, C], f32, name=f"gath{gi}")
        for (n0, nsz) in ((0, 512), (512, 256)):
            pg = pp.tile([P, 512], f32, space="PSUM", name="ps")
            for t, tsz in ((0, P), (1, 68)):
                nc.tensor.matmul(pg[0:mssz, 0:nsz],
                                 lhsT=PTf[0:tsz, t, ms0:ms0 + mssz],
                                 rhs=x_nc[0:tsz, t, n0:n0 + nsz],
                                 start=(t == 0), stop=(t == 1))
            nc.vector.tensor_copy(gt[0:mssz, n0:n0 + nsz], pg[0:mssz, 0:nsz])
        gath.append(gt)
    xo0, xo1, ntk = gath

    # feature-major views for the distance matmul via PE transposes (exact)
    xntT = pb.tile([P, 6, 196], f32, name="xntT")
    for ci in range(6):
        ptr = pp.tile([P, 196], f32, space="PSUM", name="ps")
        for (src_t, r0, rsz) in ((xo0, 0, P), (xo1, P, 10), (ntk, L, M)):
            nc.tensor.transpose(out=ptr[:, r0:r0 + rsz],
                                in_=src_t[0:rsz, ci * P:(ci + 1) * P],
                                identity=ident[0:rsz, 0:rsz])
        nc.vector.tensor_copy(xntT[:, ci, :], ptr[:])

    # column norms of gathered kept tokens -> 1/||xo_l|| (sqt reuses the PTf
    # slot: the attnG matmuls and the PT cast above are PTf's last readers)
    sqt = pb.tile([P, 6, L], f32, name="PTf")
    nc.vector.tensor_tensor(out=sqt[:], in0=xntT[:, :, 0:L], in1=xntT[:, :, 0:L],
                            op=OP.mult)
    pn = pp.tile([1, L], f32, space="PSUM", name="ps")
    for ci in range(6):
        nc.tensor.matmul(pn[:], lhsT=onesP[:, 0:1], rhs=sqt[:, ci, :],
                         start=(ci == 0), stop=(ci == 5))
    invxo = pb.tile([1, L], f32, name="invxo")
    dve_rsqrt(nc, pb, invxo[:], pn[:], [1, L], "nx")

    # raw distance [M, L] and scaled version for argmax
    pr = pp.tile([M, L], f32, space="PSUM", name="ps")
    for ci in range(6):
        nc.tensor.matmul(pr[:], lhsT=xntT[:, ci, L:196], rhs=xntT[:, ci, 0:L],
                         start=(ci == 0), stop=(ci == 5))
    praw = pb.tile([M, L], f32, name="praw")
    nc.vector.tensor_copy(praw[:], pr[:])
    pbc = pp.tile([M, L], f32, space="PSUM", name="ps")
    nc.tensor.matmul(pbc[:], lhsT=onesRow[0:1, 0:M], rhs=invxo[:],
                     start=True, stop=True)
    nc.vector.tensor_tensor(out=praw[:], in0=praw[:], in1=pbc[:], op=OP.mult)

    # argmax (first occurrence) -> one-hot firsthot [M, L]; praw holds the
    # scaled distances and is consumed in place
    rmax = pb.tile([M, 1], f32, name="rmax")
    nc.vector.tensor_reduce(out=rmax[:], in_=praw[:], axis=AX.X, op=OP.max)
    nc.vector.tensor_scalar(out=praw[:], in0=praw[:], scalar1=rmax[:, 0:1],
                            scalar2=None, op0=OP.is_equal)
    nc.vector.tensor_tensor(out=praw[:], in0=praw[:], in1=IotaLmB[0:M, :],
                            op=OP.mult)
    nc.vector.tensor_scalar(out=praw[:], in0=praw[:], scalar1=65536.0,
                            scalar2=None, op0=OP.add)
    mina = pb.tile([M, 1], f32, name="mina")
    nc.vector.tensor_reduce(out=mina[:], in_=praw[:], axis=AX.X, op=OP.min)
    fh = pb.tile([M, L], f32, name="fh")
    nc.vector.tensor_scalar(out=fh[:], in0=IotaF[0:M, 0:L], scalar1=mina[:, 0:1],
                            scalar2=None, op0=OP.is_equal)

    # weighted tokens (f32r: scatter feeds output values only, not selection)
    ntw = pb.tile([M, C], f32r, name="ntw")
    nc.vector.tensor_scalar(out=ntw[:], in0=ntk[0:M, :], scalar1=attnG[2][0:M, 0:1],
                            scalar2=None, op0=OP.mult)
    fhr = pb.tile([M, L], f32r, name="fhr")
    nc.vector.tensor_copy(fhr[:], fh[:])

    # scatter-add into kept rows + divide by merged attention
    for ki_, (ms0, mssz) in enumerate(((0, P), (P, 10))):
        pd = pp.tile([P, 1], f32, space="PSUM", name="ps")
        nc.tensor.matmul(pd[0:mssz, :], lhsT=fh[:, ms0:ms0 + mssz],
                         rhs=attnG[2][0:M, 0:1], start=True, stop=True)
        dsum = pb.tile([P, 1], f32, name=f"dsum{ki_}")
        nc.vector.tensor_tensor(out=dsum[0:mssz, :], in0=attnG[ki_][0:mssz, :],
                                in1=pd[0:mssz, :], op=OP.add)
        rd = pb.tile([P, 1], f32, name=f"rd{ki_}")
        newton_recip(nc, pb, rd[0:mssz, :], dsum[0:mssz, :], [mssz, 1], f"d{ki_}")
        xow = (xo0, xo1)[ki_]  # weighted in place; the gathered rows are dead
        nc.vector.tensor_scalar(out=xow[0:mssz, :], in0=xow[0:mssz, :],
                                scalar1=attnG[ki_][0:mssz, 0:1], scalar2=None,
                                op0=OP.mult)
        for (n0, nsz) in ((0, 512), (512, 256)):
            ps = pp.tile([P, 512], f32, space="PSUM", name="ps")
            nc.tensor.matmul(ps[0:mssz, 0:nsz], lhsT=fhr[:, ms0:ms0 + mssz],
                             rhs=ntw[:, n0:n0 + nsz], start=True, stop=True)
            nc.vector.tensor_tensor(out=xow[0:mssz, n0:n0 + nsz],
                                    in0=xow[0:mssz, n0:n0 + nsz],
                                    in1=ps[0:mssz, 0:nsz], op=OP.add)
        nc.vector.tensor_scalar(out=xow[0:mssz, :], in0=xow[0:mssz, :],
                                scalar1=rd[0:mssz, 0:1], scalar2=None, op0=OP.mult)
        nc.sync.dma_start(
            out=xsel_d[samp * NO + 1 + ms0: samp * NO + 1 + ms0 + mssz, :],
            in_=xow[0:mssz, :])


def _phase_c_chunk(nc, tc, pcs, pp, t0, tsz, xsel_d, fc1_r, fc2_r, ident,
                   out_flat, pc1):
    tiles = _ceil_tiles(tsz)
    nt = len(tiles)
    xc_sb = pcs.tile([P, 4, C], f32, name="xc_sb")
    xn2 = pc1.tile([P, 4, C], f32, name="xn2")
    for ti, (m0, msz) in enumerate(tiles):
        nc.sync.dma_start(out=xc_sb[0:msz, ti, :],
                          in_=xsel_d[t0 + m0: t0 + m0 + msz, :])
        layernorm_tile(nc, pcs, pp, xn2[0:msz, ti, :], xc_sb[0:msz, ti, :],
                       msz, False, "c")
    xnT = pc1.tile([P, 6, 512], bf16, name="xnT2")
    for ci in range(6):
        ptr = pp.tile([P, 512], f32, space="PSUM", name="ps")
        for ti, (m0, msz) in enumerate(tiles):
            nc.tensor.transpose(out=ptr[:, m0:m0 + msz],
                                in_=xn2[0:msz, ti, ci * P:(ci + 1) * P],
                                identity=ident[0:msz, 0:msz])
        nc.vector.tensor_copy(xnT[:, ci, 0:tsz], ptr[:, 0:tsz])

    # fc1 (bf16, resident) + gelu -> hT [H4, tsz] feature-major
    hT = pc1.tile([P, 24, 512], bf16, name="hT")
    for oi in range(24):
        pf = pp.tile([P, 512], f32, space="PSUM", name="ps")
        for ki in range(6):
            nc.tensor.matmul(pf[:, 0:tsz], lhsT=fc1_r[:, ki, oi * P:(oi + 1) * P],
                             rhs=xnT[:, ki, 0:tsz], start=(ki == 0), stop=(ki == 5))
        nc.scalar.activation(hT[:, oi, 0:tsz], pf[:, 0:tsz], AF.Gelu,
                             bias=KC["zero"][:, 0:1])

    # fc2 (bf16) + residual -> out
    for ti, (m0, msz) in enumerate(tiles):
        for (n0, nsz) in ((0, 512), (512, 256)):
            pf = pp.tile([P, 512], f32, space="PSUM", name="ps")
            for ki in range(24):
                nc.tensor.matmul(pf[0:msz, 0:nsz],
                                 lhsT=hT[:, ki, m0:m0 + msz],
                                 rhs=fc2_r[:, ki, n0:n0 + nsz],
                                 start=(ki == 0), stop=(ki == 23))
            nc.vector.tensor_tensor(out=xc_sb[0:msz, ti, n0:n0 + nsz],
                                    in0=xc_sb[0:msz, ti, n0:n0 + nsz],
                                    in1=pf[0:msz, 0:nsz], op=OP.add)
        nc.sync.dma_start(out=out_flat[t0 + m0: t0 + m0 + msz, :],
                          in_=xc_sb[0:msz, ti, :])


_NC_CACHE = None


def kernel(**inputs):
    global _NC_CACHE
    if _NC_CACHE is None:
        _NC_CACHE = build_nc()
    nc = _NC_CACHE

    import ml_dtypes
    x = np.ascontiguousarray(np.asarray(inputs["x"], dtype=np.float32))
    wnames = ["qkv_w", "proj_w", "fc1_w", "fc2_w"]
    ws = {k: np.ascontiguousarray(np.asarray(inputs[k], dtype=np.float32))
          for k in wnames}
    ws["fc1_w"] = ws["fc1_w"].astype(ml_dtypes.bfloat16)
    ws["fc2_w"] = ws["fc2_w"].astype(ml_dtypes.bfloat16)
    B = x.shape[0]
    n_cores = 8
    bl = B // n_cores
    in_maps = []
    for c in range(n_cores):
        m = {"x": x[c * bl:(c + 1) * bl]}
        m.update(ws)
        in_maps.append(m)
    res = run_bass_kernel_spmd(nc, in_maps, core_ids=list(range(n_cores)))
    out = np.concatenate([r["out"] for r in res.results], axis=0)
    return out.astype(np.float32)


# revision 27
# speedup vs baseline: 1.0175x; 1.0175x over previous
"""TRN2 Bass kernel for nn_Block_18227841204857 (EViT-style block with top-k token
merging). Data-parallel over batch: 8 cores x 16 samples.

Contract: kernel(**inputs) takes full unsharded inputs, returns full output
(128, 139, 768) float32.

Precision strategy: the top-k selection path (ln1, k-projection, cls-query q0,
cls scores, softmax-mean) stays true fp32; everything else off the selection
path runs f32r matmuls (1 cycle/row at free>=256 vs fp32's 4) or bf16 (MLP).
"""
import sys
sys.path.insert(0, "/opt/trn_rl_repo")

import math
import numpy as np

import concourse.bacc as bacc
import concourse.bass as bass
import concourse.mybir as mybir
from concourse.tile import TileContext
from concourse.masks import make_identity
from concourse.bass_utils import run_bass_kernel_spmd

P = 128
f32 = mybir.dt.float32
f32r = mybir.dt.float32r
bf16 = mybir.dt.bfloat16
i32 = mybir.dt.int32
AF = mybir.ActivationFunctionType
OP = mybir.AluOpType
AX = mybir.AxisListType

B_L = 16          # samples per core
N = 197           # tokens
C = 768           # channels
NH = 12           # heads
HD = 64           # head dim
L = 138           # kept tokens
M = 58            # pruned tokens
NO = 139          # output tokens (cls + kept)
H4 = 3072         # mlp hidden
EPS = 1e-5
NCHUNK = 8        # phase-A chunks (2 samples each)
T2 = 2 * N        # 394 tokens per chunk
QPAD = 256        # free size for f32r score matmuls (>=256 for 1 cycle/row)

LOG2E = float(np.float32(1.4426950408889634))
LN2 = float(np.float32(0.6931471805599453))
MAGIC = 12582912.0  # 1.5 * 2**23, round-to-nearest-int trick

KC = {}  # const tiles shared across build helpers


def _sample_tiles():
    # token tiles within one sample: (tile_idx, start, size)
    return [(0, 0, 128), (1, 128, 69)]


def _ceil_tiles(n):
    out = []
    s = 0
    while s < n:
        sz = min(P, n - s)
        out.append((s, sz))
        s += sz
    return out


def dve_exp(nc, sb, out_ap, in_ap, rows, cols,
            names=("exp_y", "exp_nf", "exp_t", "exp_p", "exp_ni")):
    """out = exp(in) elementwise, ~3e-7 rel accuracy, DVE+ACT only.

    exp(s) = 2^n * e^f,  n = round(s*log2e),  f = s - n*ln2  in [-0.347, 0.347]
    names lets callers alias the temporaries onto dead slots in their pool.
    """
    shp = [rows, cols]
    y = sb.tile(shp, f32, name=names[0])
    nf = sb.tile(shp, f32, name=names[1])
    t = sb.tile(shp, f32, name=names[2])
    p = sb.tile(shp, f32, name=names[3])
    u = y  # y is dead once nf is extracted; reuse as Horner ping-pong
    ni = sb.tile(shp, i32, name=names[4])

    nc.vector.tensor_scalar(out=y[:], in0=in_ap, scalar1=LOG2E, scalar2=MAGIC,
                            op0=OP.mult, op1=OP.add)
    nc.vector.tensor_scalar(out=nf[:], in0=y[:], scalar1=MAGIC, scalar2=None,
                            op0=OP.subtract)
    # t = s - nf*ln2   (scalar_tensor_tensor: (in0 op0 scalar) op1 in1)
    nc.vector.scalar_tensor_tensor(out=t[:], in0=nf[:], scalar=-LN2, in1=in_ap,
                                   op0=OP.mult, op1=OP.add)
    # Horner for e^t, t in [-0.35, 0.35]: coeffs 1/k!, k=8..0
    coef = [1.0 / math.factorial(k) for k in range(9)]
    nc.vector.tensor_scalar(out=p[:], in0=t[:], scalar1=coef[8], scalar2=coef[7],
                            op0=OP.mult, op1=OP.add)
    for k in range(6, -1, -1):
        nc.vector.tensor_tensor(out=u[:], in0=p[:], in1=t[:], op=OP.mult)
        nc.vector.tensor_scalar(out=p[:], in0=u[:], scalar1=coef[k], scalar2=None,
                                op0=OP.add)
    # 2^n via bit tricks: (n + 127) << 23, bitcast to f32
    nc.vector.tensor_copy(ni[:], nf[:])
    nc.vector.tensor_scalar(out=ni[:], in0=ni[:], scalar1=127, scalar2=None, op0=OP.add)
    nc.vector.tensor_scalar(out=ni[:], in0=ni[:], scalar1=23, scalar2=None,
                            op0=OP.arith_shift_left)
    nc.vector.tensor_tensor(out=out_ap, in0=p[:], in1=ni[:].bitcast(f32), op=OP.mult)


def newton_recip(nc, sb, out_ap, in_ap, shp, tag):
    """out = 1/in with one Newton refinement on DVE reciprocal."""
    r0 = sb.tile(shp, f32, name=f"nr_r0_{tag}")
    tt = sb.tile(shp, f32, name=f"nr_t_{tag}")
    nc.vector.reciprocal(r0[:], in_ap)
    nc.vector.tensor_tensor(out=tt[:], in0=r0[:], in1=in_ap, op=OP.mult)
    nc.vector.tensor_scalar(out=tt[:], in0=tt[:], scalar1=-1.0, scalar2=2.0,
                            op0=OP.mult, op1=OP.add)
    nc.vector.tensor_tensor(out=out_ap, in0=r0[:], in1=tt[:], op=OP.mult)


def dve_rsqrt(nc, sb, out_ap, in_ap, shp, tag):
    """out = 1/sqrt(in) on DVE only (quake seed + 3 Newton iters)."""
    v = sb.tile(shp, f32, name=f"rq_v_{tag}")
    yb = sb.tile(shp, i32, name=f"rq_yb_{tag}")
    t1 = sb.tile(shp, f32, name=f"rq_t1_{tag}")
    nc.vector.tensor_copy(v[:], in_ap)
    nc.vector.tensor_scalar(out=yb[:], in0=v[:].bitcast(i32), scalar1=1,
                            scalar2=None, op0=OP.arith_shift_right)
    nc.vector.tensor_scalar(out=yb[:], in0=yb[:], scalar1=-1,
                            scalar2=0x5F3759DF, op0=OP.mult, op1=OP.add)
    y = yb[:].bitcast(f32)
    for _ in range(3):
        nc.vector.tensor_tensor(out=t1[:], in0=v[:], in1=y, op=OP.mult)
        nc.vector.tensor_tensor(out=t1[:], in0=t1[:], in1=y, op=OP.mult)
        nc.vector.tensor_scalar(out=t1[:], in0=t1[:], scalar1=-0.5, scalar2=1.5,
                                op0=OP.mult, op1=OP.add)
        nc.vector.tensor_tensor(out=out_ap, in0=y, in1=t1[:], op=OP.mult)
        y = out_ap


def layernorm_tile(nc, sb, ps, out_ap, x_ap, rows, newton, tag):
    """LayerNorm over free dim (768), w=1 b=0. out/x: [rows, C]."""
    shp = [rows, 1]
    rs = sb.tile(shp, f32, name=f"ln_rs_{tag}")
    nm = sb.tile(shp, f32, name=f"ln_nm_{tag}")
    xc = sb.tile([rows, C], f32, name=f"ln_xc_{tag}")
    sq = sb.tile([rows, C], f32, name=f"ln_sq_{tag}")
    vs = sb.tile(shp, f32, name=f"ln_vs_{tag}")
    sd = sb.tile(shp, f32, name=f"ln_sd_{tag}")
    nc.vector.tensor_reduce(out=rs[:], in_=x_ap, axis=AX.X, op=OP.add)
    nc.vector.tensor_scalar(out=nm[:], in0=rs[:], scalar1=-1.0 / C, scalar2=None,
                            op0=OP.mult)
    nc.vector.tensor_scalar(out=xc[:], in0=x_ap, scalar1=nm[:, 0:1], scalar2=None,
                            op0=OP.add)
    nc.vector.tensor_tensor(out=sq[:], in0=xc[:], in1=xc[:], op=OP.mult)
    nc.vector.tensor_reduce(out=vs[:], in_=sq[:], axis=AX.X, op=OP.add)
    nc.scalar.activation(sd[:], vs[:], AF.Sqrt, scale=1.0 / C,
                         bias=KC["eps"][0:rows, 0:1])
    inv = sb.tile(shp, f32, name=f"ln_inv_{tag}")
    if newton:
        newton_recip(nc, sb, inv[:], sd[:], shp, f"ln_{tag}")
    else:
        nc.vector.reciprocal(inv[:], sd[:])
    nc.vector.tensor_scalar(out=out_ap, in0=xc[:], scalar1=inv[:, 0:1], scalar2=None,
                            op0=OP.mult)


def build_nc():
    nc = bacc.Bacc("TRN2", target_bir_lowering=False, debug=False, num_swdge_queues=1)

    x_in = nc.declare_dram_parameter("x", [B_L, N, C], f32, isOutput=False)
    qkv_w = nc.declare_dram_parameter("qkv_w", [C, 3 * C], f32, isOutput=False)
    proj_w = nc.declare_dram_parameter("proj_w", [C, C], f32, isOutput=False)
    fc1_w = nc.declare_dram_parameter("fc1_w", [C, H4], bf16, isOutput=False)
    fc2_w = nc.declare_dram_parameter("fc2_w", [H4, C], bf16, isOutput=False)
    out_ext = nc.declare_dram_parameter("out", [B_L, NO, C], f32, isOutput=True)

    out_flat = out_ext.ap().rearrange("b n c -> (b n) c")

    with TileContext(nc) as tc:
        _build_body(nc, tc, x_in, qkv_w, proj_w, fc1_w, fc2_w, out_flat)
    nc.finalize()
    return nc


def _build_body(nc, tc, x_in, qkv_w, proj_w, fc1_w, fc2_w, out_flat):
    from contextlib import ExitStack

    ctx = ExitStack()
    with ctx:
        # ---------- constants ----------
        pc = ctx.enter_context(tc.tile_pool(name="const", bufs=1))
        pdram = ctx.enter_context(tc.tile_pool(name="dram", bufs=1, space="DRAM"))

        ident = pc.tile([P, P], f32)
        make_identity(nc, ident[:])
        onesP = pc.tile([P, 1], f32)
        nc.vector.memset(onesP[:], 1.0)
        onesRow = pc.tile([1, P], f32)
        nc.vector.memset(onesRow[:], 1.0)
        epsb = pc.tile([P, 1], f32)
        nc.vector.memset(epsb[:], EPS)
        zerob = pc.tile([P, 1], f32)
        nc.vector.memset(zerob[:], 0.0)
        KC["eps"] = epsb
        KC["zero"] = zerob

        iota_i = pc.tile([P, 1], i32)
        nc.gpsimd.iota(iota_i[:], pattern=[[1, 1]], base=0, channel_multiplier=1)
        iotaPf = pc.tile([P, 1], f32)
        nc.vector.tensor_copy(iotaPf[:], iota_i[:])
        iotaPf128 = pc.tile([P, 1], f32)
        nc.vector.tensor_scalar(out=iotaPf128[:], in0=iotaPf[:], scalar1=128.0,
                                scalar2=None, op0=OP.add)

        iota_f_i = pc.tile([P, 196], i32)
        nc.gpsimd.iota(iota_f_i[:], pattern=[[1, 196]], base=0, channel_multiplier=0)
        IotaF = pc.tile([P, 196], f32)
        nc.vector.tensor_copy(IotaF[:], iota_f_i[:])
        IotaLmB = pc.tile([P, L], f32)
        nc.vector.tensor_scalar(out=IotaLmB[:], in0=IotaF[:, 0:L], scalar1=-65536.0,
                                scalar2=None, op0=OP.add)

        LT0 = pc.tile([P, 196], f32)
        nc.vector.tensor_scalar(out=LT0[:], in0=IotaF[:], scalar1=iotaPf[:, 0:1],
                                scalar2=None, op0=OP.is_lt)
        LT1 = pc.tile([P, 196], f32)
        nc.vector.tensor_scalar(out=LT1[:], in0=IotaF[:], scalar1=iotaPf128[:, 0:1],
                                scalar2=None, op0=OP.is_lt)
        TRI0 = pc.tile([P, 196], f32)
        nc.vector.tensor_scalar(out=TRI0[:], in0=IotaF[:], scalar1=iotaPf[:, 0:1],
                                scalar2=None, op0=OP.is_ge)
        TRI1 = pc.tile([P, 196], f32)
        nc.vector.tensor_scalar(out=TRI1[:], in0=IotaF[:], scalar1=iotaPf128[:, 0:1],
                                scalar2=None, op0=OP.is_ge)

        # ---------- DRAM scratch ----------
        xattn_d = pdram.tile([B_L * N, C], f32)
        xsel_d = pdram.tile([B_L * NO, C], f32)
        clsD = pdram.tile([B_L, 196], f32)

        # ================= PHASE A + interleaved selection =================
        with ExitStack() as actx:
            pw = actx.enter_context(tc.tile_pool(name="aw", bufs=1))
            pp = actx.enter_context(tc.tile_pool(name="aps", bufs=6, space="PSUM"))

            # resident weights: all f32r; the fp32 selection path uses the
            # precomputed u-vectors (u_h = Wk_h @ q0_h) instead of fp32 k
            qkvKr = pw.tile([P, 6, C], f32r)
            qkvQr = pw.tile([P, 6, C], f32r)
            qkvVr = pw.tile([P, 6, C], f32r)
            projr = pw.tile([P, 6, C], f32r)
            u_all = pw.tile([P, 6, B_L * NH], f32)

            with ExitStack() as sctx:
                pstage = sctx.enter_context(tc.tile_pool(name="stg", bufs=1))
                # separate staging tiles so the three weight loads + casts all
                # run in parallel (across DMA and the three copy engines)
                wq = pstage.tile([P, 6, C], f32, name="wq")
                wv = pstage.tile([P, 6, C], f32, name="wv")
                wp = pstage.tile([P, 6, C], f32, name="wp")
                wk = pstage.tile([P, 6, C], f32, name="wk")
                nc.sync.dma_start(
                    out=wq[:],
                    in_=qkv_w.ap()[:, 0:C].rearrange("(kt p) o -> p kt o", p=P))
                nc.sync.dma_start(
                    out=wv[:],
                    in_=qkv_w.ap()[:, 2 * C:3 * C].rearrange("(kt p) o -> p kt o", p=P))
                nc.sync.dma_start(
                    out=wp[:],
                    in_=proj_w.ap().rearrange("(kt p) o -> p kt o", p=P))
                nc.sync.dma_start(
                    out=wk[:],
                    in_=qkv_w.ap()[:, C:2 * C].rearrange("(kt p) o -> p kt o", p=P))
                nc.scalar.copy(qkvVr[:], wv[:])
                nc.vector.tensor_copy(projr[:], wp[:])
                nc.scalar.copy(qkvKr[:], wk[:])
                # Wk^T (fp32, exact transposes) for the u-vector pre-pass
                wkT = pstage.tile([P, 6, C], f32, name="wkT")
                for bt in range(6):
                    ptw = pp.tile([P, P], f32, space="PSUM", name="ps")
                    for at in range(6):
                        nc.tensor.transpose(
                            out=ptw[:, 0:P],
                            in_=wk[:, at, bt * P:(bt + 1) * P],
                            identity=ident[:])
                        nc.vector.tensor_copy(wkT[:, bt, at * P:(at + 1) * P],
                                              ptw[:, 0:P])
                # --- q0 pre-pass: fp32 cls-query for all 16 samples ---
                xcls = pstage.tile([B_L, C], f32, name="xcls")
                nc.sync.dma_start(out=xcls[:], in_=x_in.ap()[:, 0, :])
                xn0 = pstage.tile([B_L, C], f32, name="xn0")
                layernorm_tile(nc, pstage, pp, xn0[:], xcls[:], B_L, True, "q0")
                xn0T = pstage.tile([P, 6, B_L], f32, name="xn0T")
                for ci in range(6):
                    ptq = pp.tile([P, B_L], f32, space="PSUM", name="ps")
                    nc.tensor.transpose(out=ptq[:, 0:B_L],
                                        in_=xn0[0:B_L, ci * P:(ci + 1) * P],
                                        identity=ident[0:B_L, 0:B_L])
                    nc.vector.tensor_copy(xn0T[:, ci, :], ptq[:])
                q0_all = pstage.tile([B_L, C], f32, name="q0_all")
                for (n0, nsz) in ((0, 512), (512, 256)):
                    psq = pp.tile([B_L, 512], f32, space="PSUM", name="ps")
                    for ki in range(6):
                        nc.tensor.matmul(psq[0:B_L, 0:nsz],
                                         lhsT=xn0T[:, ki, :],
                                         rhs=wq[:, ki, n0:n0 + nsz],
                                         start=(ki == 0), stop=(ki == 5))
                    nc.vector.tensor_copy(q0_all[:, n0:n0 + nsz], psq[0:B_L, 0:nsz])
                q0T_all = pstage.tile([P, 6, B_L], f32, name="q0T_all")
                for ci in range(6):
                    ptq = pp.tile([P, B_L], f32, space="PSUM", name="ps")
                    nc.tensor.transpose(out=ptq[:, 0:B_L],
                                        in_=q0_all[0:B_L, ci * P:(ci + 1) * P],
                                        identity=ident[0:B_L, 0:B_L])
                    nc.vector.tensor_copy(q0T_all[:, ci, :], ptq[:])
                nc.vector.tensor_copy(qkvQr[:], wq[:])
                # u[c, (samp, h)] = sum_d Wk[c, h*64+d] * q0[samp, h*64+d]
                # (fp32; replaces the fp32 k-projection in the score path)
                for h in range(NH):
                    bt, off = h // 2, (h % 2) * HD
                    for ft in range(6):
                        psu = pp.tile([P, B_L], f32, space="PSUM", name="ps")
                        nc.tensor.matmul(
                            psu[:],
                            lhsT=wkT[off:off + HD, bt, ft * P:(ft + 1) * P],
                            rhs=q0T_all[off:off + HD, bt, :],
                            start=True, stop=True)
                        dst = u_all[:, ft, :].rearrange(
                            "p (s h) -> p s h", h=NH)[:, :, h]
                        nc.vector.tensor_copy(dst, psu[:])

            # main working pools enter AFTER the staging scope exits so its
            # SBUF is reclaimed (stack LIFO)
            pa = actx.enter_context(tc.tile_pool(name="aa", bufs=1))
            pa2 = actx.enter_context(tc.tile_pool(name="aa2", bufs=1))
            pb2 = actx.enter_context(tc.tile_pool(name="b2i", bufs=1))

            # software pipeline: emit chunk ch's attention + its cls-score
            # extraction, then the PREVIOUS chunk's selection/merge work so its
            # vector-engine chain overlaps chunk ch's PE-bound matmuls.
            for ch in range(NCHUNK):
                xnT = _phase_a_chunk(nc, tc, pa, pa2, pp, ch, x_in, qkvKr,
                                     qkvQr, qkvVr, projr, ident, onesP,
                                     xattn_d, xsel_d)
                if ch > 0:
                    _phase_b_rest(nc, tc, pb2, pp, ch - 1, xattn_d, xsel_d,
                                  clsD, LT0, LT1, TRI0, TRI1, IotaF, IotaLmB,
                                  onesP, onesRow, iotaPf, iotaPf128, ident)
                _phase_b_score(nc, pb2, pp, ch, u_all, xnT)
            _phase_b_rest(nc, tc, pb2, pp, NCHUNK - 1, xattn_d, xsel_d,
                          clsD, LT0, LT1, TRI0, TRI1, IotaF, IotaLmB,
                          onesP, onesRow, iotaPf, iotaPf128, ident)

        # ================= PHASE C: MLP =================
        with ExitStack() as cctx:
            pw = cctx.enter_context(tc.tile_pool(name="cw", bufs=1))
            pcs = cctx.enter_context(tc.tile_pool(name="cc", bufs=2))
            pc1 = cctx.enter_context(tc.tile_pool(name="cc1", bufs=1))
            pp = cctx.enter_context(tc.tile_pool(name="cps", bufs=6, space="PSUM"))

            fc2_r = pw.tile([P, 24, C], bf16)
            w2view = fc2_w.ap().rearrange("(kt p) o -> p kt o", p=P)
            nc.sync.dma_start(out=fc2_r[:], in_=w2view[:])
            fc1_r = pw.tile([P, 6, H4], bf16)
            nc.sync.dma_start(out=fc1_r[:],
                              in_=fc1_w.ap().rearrange("(kt p) o -> p kt o", p=P))

            TOK = B_L * NO  # 2224
            TCK = 512
            nch = (TOK + TCK - 1) // TCK
            for ci in range(nch):
                t0 = ci * TCK
                tsz = min(TCK, TOK - t0)
                _phase_c_chunk(nc, tc, pcs, pp, t0, tsz, xsel_d, fc1_r,
                               fc2_r, ident, out_flat, pc1)


def _phase_b_score(nc, pb, pp, ch, u_all, xnT):
    """Raw cls scores for samples 2ch, 2ch+1: s[h, j] = xn_j . u_h (fp32),
    mathematically q0_h . k_{j,h} with the Wk contraction pre-folded into u."""
    s_all = pb.tile([NH, 2, N], f32, name="s_all")
    for s2 in range(2):
        samp = 2 * ch + s2
        psc = pp.tile([NH, N], f32, space="PSUM", name="ps")
        for ki in range(6):
            nc.tensor.matmul(psc[:],
                             lhsT=u_all[:, ki, samp * NH:(samp + 1) * NH],
                             rhs=xnT[:, ki, s2 * N:(s2 + 1) * N],
                             start=(ki == 0), stop=(ki == 5))
        nc.vector.tensor_scalar(out=s_all[:, s2, :], in0=psc[:],
                                scalar1=0.125, scalar2=None, op0=OP.mult)


def _phase_b_rest(nc, tc, pb, pp, ch, xattn_d, xsel_d, clsD,
                  LT0, LT1, TRI0, TRI1, IotaF, IotaLmB, onesP, onesRow,
                  iotaPf, iotaPf128, ident):
    """Softmax + topk rank + token merge for chunk ch, emitted one chunk late
    so its vector-engine chain overlaps the next chunk's PE-bound matmuls."""
    s_all = pb.tile([NH, 2, N], f32, name="s_all")
    # max-subtracted exp + softmax + head mean
    smax = pb.tile([NH, 2], f32, name="smax")
    nc.vector.tensor_reduce(out=smax[:], in_=s_all[:], axis=AX.X, op=OP.max)
    sbc = bass.AP(tensor=smax[:].tensor, offset=smax[:].offset,
                  ap=[[smax[:].ap[0][0], NH], [1, 2], [0, N]])
    nc.vector.tensor_tensor(out=s_all[:], in0=s_all[:], in1=sbc, op=OP.subtract)
    nc.vector.tensor_scalar(out=s_all[:], in0=s_all[:], scalar1=-80.0,
                            scalar2=None, op0=OP.max)
    e_all = pb.tile([NH, 2, N], f32, name="e_all")
    # temporaries alias phase-b2 slots that are dead at this point in the chunk
    dve_exp(nc, pb, e_all[:].rearrange("h s n -> h (s n)"),
            s_all[:].rearrange("h s n -> h (s n)"), NH, 2 * N,
            names=("gath0", "gath1", "gath2", "x_nc", "xntT"))
    den = pb.tile([NH, 2], f32, name="den")
    nc.vector.tensor_reduce(out=den[:], in_=e_all[:], axis=AX.X, op=OP.add)
    rden = smax  # smax is dead after the subtract above
    newton_recip(nc, pb, rden[:], den[:], [NH, 2], "den")
    a_all = e_all  # normalize in place; raw exps are dead after this
    rbc = bass.AP(tensor=rden[:].tensor, offset=rden[:].offset,
                  ap=[[rden[:].ap[0][0], NH], [1, 2], [0, N]])
    nc.vector.tensor_tensor(out=a_all[:], in0=e_all[:], in1=rbc, op=OP.mult)
    cls_all = pb.tile([1, 2, 196], f32, name="cls_all")
    for s2 in range(2):
        pcm = pp.tile([1, 196], f32, space="PSUM", name="ps")
        nc.tensor.matmul(pcm[:], lhsT=onesP[0:NH, 0:1],
                         rhs=a_all[:, s2, 1:N], start=True, stop=True)
        nc.vector.tensor_scalar(out=cls_all[0:1, s2, :], in0=pcm[:],
                                scalar1=1.0 / 12.0, scalar2=None, op0=OP.mult)

    # per-key cls values, [i(2 tiles), s2]
    clsPT = pb.tile([P, 2, 2], f32, name="clsPT")
    for s2 in range(2):
        ptt = pp.tile([P, 1], f32, space="PSUM", name="ps")
        nc.tensor.transpose(out=ptt[:, 0:1], in_=cls_all[0:1, s2, 0:P],
                            identity=ident[0:1, 0:1])
        nc.vector.tensor_copy(clsPT[:, 0, s2:s2 + 1], ptt[:, 0:1])
        ptt2 = pp.tile([P, 1], f32, space="PSUM", name="ps")
        nc.tensor.transpose(out=ptt2[0:68, 0:1], in_=cls_all[0:1, s2, P:196],
                            identity=ident[0:1, 0:1])
        nc.vector.tensor_copy(clsPT[0:68, 1, s2:s2 + 1], ptt2[0:68, 0:1])

    # ranks, one sample at a time (VF: cls row broadcast to all partitions via
    # a rank-1 PE matmul into PSUM — avoids a DRAM round-trip on the critical
    # chain; the compare ops read the psum directly)
    rank_all = pb.tile([P, 2, 2], f32, name="rank_all")
    pruned_all = pb.tile([P, 2, 2], f32, name="pruned_all")
    for s2 in range(2):
        VFp = pp.tile([P, 196], f32, space="PSUM", name="ps")
        nc.tensor.matmul(VFp[:], lhsT=onesRow[0:1, 0:P],
                         rhs=cls_all[0:1, s2, :], start=True, stop=True)
        VF = pb.tile([P, 196], f32, name="VF")
        nc.scalar.copy(VF[:], VFp[:])
        for t, lt in ((0, LT0), (1, LT1)):
            src = clsPT[:, t, s2:s2 + 1]
            vp = bass.AP(tensor=src.tensor, offset=src.offset,
                         ap=[src.ap[0], [0, 196]])
            gt = pb.tile([P, 196], f32, name="rk_gt")
            eq = pb.tile([P, 196], f32, name="rk_eq")
            nc.vector.tensor_tensor(out=gt[:], in0=VF[:], in1=vp, op=OP.is_gt)
            nc.vector.tensor_tensor(out=eq[:], in0=VF[:], in1=vp, op=OP.is_equal)
            nc.vector.tensor_tensor(out=eq[:], in0=eq[:], in1=lt[:], op=OP.mult)
            nc.vector.tensor_tensor(out=gt[:], in0=gt[:], in1=eq[:], op=OP.add)
            nc.vector.tensor_reduce(out=rank_all[:, t, s2:s2 + 1], in_=gt[:],
                                    axis=AX.X, op=OP.add)
            nc.vector.tensor_scalar(out=pruned_all[:, t, s2:s2 + 1],
                                    in0=rank_all[:, t, s2:s2 + 1],
                                    scalar1=137.5, scalar2=None, op0=OP.is_gt)
    # inclusive cumsum of pruned -> pos
    posP = pb.tile([P, 2, 2], f32, name="posP")
    pp0 = pp.tile([P, 2], f32, space="PSUM", name="ps")
    nc.tensor.matmul(pp0[:], lhsT=TRI0[:, 0:P], rhs=pruned_all[:, 0, :],
                     start=True, stop=False)
    nc.tensor.matmul(pp0[:], lhsT=TRI1[0:68, 0:P], rhs=pruned_all[0:68, 1, :],
                     start=False, stop=True)
    nc.vector.tensor_copy(posP[:, 0, :], pp0[:])
    pp1 = pp.tile([P, 2], f32, space="PSUM", name="ps")
    nc.tensor.matmul(pp1[0:68, :], lhsT=TRI0[:, P:196],
                     rhs=pruned_all[:, 0, :], start=True, stop=False)
    nc.tensor.matmul(pp1[0:68, :], lhsT=TRI1[0:68, P:196],
                     rhs=pruned_all[0:68, 1, :], start=False, stop=True)
    nc.vector.tensor_copy(posP[0:68, 1, :], pp1[0:68, :])

    for s2 in range(2):
        _phase_b2_sample(nc, tc, pb, pp, 2 * ch + s2, s2, xattn_d, xsel_d,
                         rank_all, pruned_all, posP, clsPT, IotaF, IotaLmB,
                         onesP, onesRow, iotaPf, iotaPf128, ident)


def _phase_a_chunk(nc, tc, pa, pa2, pp, ch, x_in, qkvKr, qkvQr, qkvVr, projr,
                   ident, onesP, xattn_d, xsel_d):
    st = _sample_tiles()
    x_sb = pa2.tile([P, 2, 2, C], f32, name="x_sb")
    xn_sb = pa.tile([P, 2, 2, C], f32, name="xn_sb")
    for s2 in range(2):
        samp = 2 * ch + s2
        for (mt, m0, msz) in st:
            nc.sync.dma_start(out=x_sb[0:msz, s2, mt, :],
                              in_=x_in.ap()[samp, m0:m0 + msz, :])
            layernorm_tile(nc, pa, pp, xn_sb[0:msz, s2, mt, :],
                           x_sb[0:msz, s2, mt, :], msz, True, "a")

    # transpose ln1 out -> feature-major [C, T2]: fp32 + f32r twin copies
    xnT = pa.tile([P, 6, T2], f32, name="xnT")
    xnTr = pa.tile([P, 6, T2], f32r, name="xnTr")
    for ci in range(6):
        ptr = pp.tile([P, T2], f32, space="PSUM", name="ps")
        for s2 in range(2):
            for (mt, m0, msz) in st:
                nc.tensor.transpose(
                    out=ptr[:, s2 * N + m0: s2 * N + m0 + msz],
                    in_=xn_sb[0:msz, s2, mt, ci * P:(ci + 1) * P],
                    identity=ident[0:msz, 0:msz])
        nc.vector.tensor_copy(xnT[:, ci, :], ptr[:])
        nc.scalar.copy(xnTr[:, ci, :], ptr[:])

    # q feature-major f32r
    qTr = pa.tile([P, 6, T2], f32r, name="qTr")
    for oi in range(6):
        pq = pp.tile([P, T2], f32, space="PSUM", name="ps")
        for ki in range(6):
            nc.tensor.matmul(
                pq[:],
                lhsT=qkvQr[:, ki, oi * P:(oi + 1) * P],
                rhs=xnTr[:, ki, :], start=(ki == 0), stop=(ki == 5))
        nc.vector.tensor_copy(qTr[:, oi, :], pq[:])

    # v token-major with leading ones column per head: [tok, (12, 65)]
    v_blk = pa.tile([P, 2, 2, NH, 65], f32, name="v_blk")
    nc.vector.memset(v_blk[:, :, :, :, 0:1], 1.0)
    for s2 in range(2):
        for (mt, m0, msz) in st:
            for nc_i, (n0, nsz) in enumerate(((0, 512), (512, 256))):
                pv = pp.tile([P, 512], f32, space="PSUM", name="ps")
                for ki in range(6):
                    nc.tensor.matmul(
                        pv[0:msz, 0:nsz],
                        lhsT=xnTr[:, ki, s2 * N + m0: s2 * N + m0 + msz],
                        rhs=qkvVr[:, ki, n0:n0 + nsz],
                        start=(ki == 0), stop=(ki == 5))
                h0 = n0 // HD
                nhh = nsz // HD
                nc.vector.tensor_copy(v_blk[0:msz, s2, mt, h0:h0 + nhh, 1:65],
                                      pv[0:msz, 0:nsz].rearrange(
                                          "p (h d) -> p h d", d=HD))

    # k feature-major f32r (the fp32 selection path no longer needs k — the
    # scores use the u-vectors against fp32 xnT). kTr reuses xnTr's slot
    # (the v section above is xnTr's last reader).
    kTr = pa.tile([P, 6, T2], f32r, name="xnTr")
    for oi in range(6):
        pq = pp.tile([P, T2], f32, space="PSUM", name="ps")
        for ki in range(6):
            nc.tensor.matmul(
                pq[:],
                lhsT=qkvKr[:, ki, oi * P:(oi + 1) * P],
                rhs=xnTr[:, ki, :], start=(ki == 0), stop=(ki == 5))
        nc.scalar.copy(kTr[:, oi, :], pq[:])

    # attention per head: scoresT (f32r, padded free) -> exp -> AV (+denom via
    # ones col) -> scale
    eT = pa.tile([P, 2, 2, N], f32, name="eT_xa")
    attn_out = pa.tile([P, 2, 2, C], f32, name="xn_sb")
    rr = pa.tile([P, 2, 2, NH], f32, name="rr")
    for h in range(NH):
        ci, off = h // 2, (h % 2) * HD
        for s2 in range(2):
            # 256-wide window keeps the f32r matmul at 1 cycle/row; for s2=1
            # the window is right-aligned (cols 138:394), so query j lands at
            # psum column 59+j instead of j.
            w0 = 0 if s2 == 0 else T2 - QPAD
            qo = s2 * N - w0
            for (nkt, k0, ksz) in st:
                psc = pp.tile([P, QPAD], f32, space="PSUM", name="ps")
                nc.tensor.matmul(
                    psc[0:ksz, :],
                    lhsT=kTr[off:off + HD, ci, s2 * N + k0: s2 * N + k0 + ksz],
                    rhs=qTr[off:off + HD, ci, w0: w0 + QPAD],
                    start=True, stop=True)
                nc.scalar.activation(eT[0:ksz, s2, nkt, :],
                                     psc[0:ksz, qo: qo + N],
                                     AF.Exp, scale=0.125,
                                     bias=KC["zero"][0:ksz, 0:1])
        for s2 in range(2):
            for (qt, q0, qsz) in st:
                po = pp.tile([P, 65], f32, space="PSUM", name="ps")
                for (nkt, k0, ksz) in st:
                    nc.tensor.matmul(
                        po[0:qsz, :],
                        lhsT=eT[0:ksz, s2, nkt, q0:q0 + qsz],
                        rhs=v_blk[0:ksz, s2, nkt, h, :],
                        start=(nkt == 0), stop=(nkt == 1))
                nc.vector.reciprocal(rr[0:qsz, s2, qt, h:h + 1], po[0:qsz, 0:1])
                nc.vector.tensor_scalar(
                    out=attn_out[0:qsz, s2, qt, h * HD:(h + 1) * HD],
                    in0=po[0:qsz, 1:65],
                    scalar1=rr[0:qsz, s2, qt, h:h + 1],
                    scalar2=None, op0=OP.mult)

    # transpose attn_out -> feature-major f32r (reuses qTr's slot: the score
    # matmuls above are qTr's last readers)
    aoTr = pa.tile([P, 6, T2], f32r, name="qTr")
    for ci in range(6):
        ptr = pp.tile([P, T2], f32, space="PSUM", name="ps")
        for s2 in range(2):
            for (mt, m0, msz) in st:
                nc.tensor.transpose(
                    out=ptr[:, s2 * N + m0: s2 * N + m0 + msz],
                    in_=attn_out[0:msz, s2, mt, ci * P:(ci + 1) * P],
                    identity=ident[0:msz, 0:msz])
        nc.scalar.copy(aoTr[:, ci, :], ptr[:])

    # proj (f32r) + residual -> xattn
    xa_sb = pa.tile([P, 2, 2, C], f32, name="eT_xa")
    for s2 in range(2):
        samp = 2 * ch + s2
        for (mt, m0, msz) in st:
            for (n0, nsz) in ((0, 512), (512, 256)):
                pj = pp.tile([P, 512], f32, space="PSUM", name="ps")
                for ki in range(6):
                    nc.tensor.matmul(
                        pj[0:msz, 0:nsz],
                        lhsT=aoTr[:, ki, s2 * N + m0: s2 * N + m0 + msz],
                        rhs=projr[:, ki, n0:n0 + nsz],
                        start=(ki == 0), stop=(ki == 5))
                nc.vector.tensor_tensor(out=xa_sb[0:msz, s2, mt, n0:n0 + nsz],
                                        in0=x_sb[0:msz, s2, mt, n0:n0 + nsz],
                                        in1=pj[0:msz, 0:nsz], op=OP.add)
            nc.sync.dma_start(out=xattn_d[samp * N + m0: samp * N + m0 + msz, :],
                              in_=xa_sb[0:msz, s2, mt, :])
        # cls row into xsel
        nc.sync.dma_start(out=xsel_d[samp * NO: samp * NO + 1, :],
                          in_=xa_sb[0:1, s2, 0, :])
    return xnT


def _phase_b2_sample(nc, tc, pb, pp, samp, s2, xattn_d, xsel_d, rank_all,
                     pruned_all, posP, clsPT, IotaF, IotaLmB, onesP, onesRow,
                     iotaPf, iotaPf128, ident):
    s = s2
    # one-hot selection matrices PT_cat [i, 196]: cols 0:138 keep, 138:196 compl
    # (f32r: exact 0/1 values, feeds the f32r gather matmuls)
    PTf = pb.tile([P, 2, 196], f32, name="PTf")
    for t, tsz in ((0, P), (1, 68)):
        nc.vector.tensor_scalar(out=PTf[0:tsz, t, 0:L], in0=IotaF[0:tsz, 0:L],
                                scalar1=rank_all[0:tsz, t, s:s + 1],
                                scalar2=None, op0=OP.is_equal)
        nc.vector.tensor_scalar(out=PTf[0:tsz, t, L:196], in0=IotaF[0:tsz, 1:59],
                                scalar1=posP[0:tsz, t, s:s + 1],
                                scalar2=None, op0=OP.is_equal)
        nc.vector.tensor_scalar(out=PTf[0:tsz, t, L:196], in0=PTf[0:tsz, t, L:196],
                                scalar1=pruned_all[0:tsz, t, s:s + 1],
                                scalar2=None, op0=OP.mult)

    mslices = ((0, P), (P, 10), (L, M))   # keep0, keep1, compl
    # gather cls_attn values (fp32: tiny)
    attnG = []
    for (ms0, mssz) in mslices:
        pg = pp.tile([P, 1], f32, space="PSUM", name="ps")
        for t, tsz in ((0, P), (1, 68)):
            nc.tensor.matmul(pg[0:mssz, :], lhsT=PTf[0:tsz, t, ms0:ms0 + mssz],
                             rhs=clsPT[0:tsz, t, s:s + 1],
                             start=(t == 0), stop=(t == 1))
        ga = pb.tile([P, 1], f32, name=f"attn_g{ms0}")
        nc.vector.tensor_copy(ga[0:mssz, :], pg[0:mssz, :])
        attnG.append(ga)

    # load non-cls rows and gather selected rows via exact one-hot matmuls (f32r)
    x_nc = pb.tile([P, 2, C], f32, name="x_nc")
    nc.sync.dma_start(out=x_nc[:, 0, :],
                      in_=xattn_d[samp * N + 1: samp * N + 129, :])
    nc.sync.dma_start(out=x_nc[0:68, 1, :],
                      in_=xattn_d[samp * N + 129: samp * N + 197, :])
    gath = []
    for gi, (ms0, mssz) in enumerate(mslices):
        gt = pb.tile([P, C], f32, name=f"gath{gi}")
        for (n0, nsz) in ((0, 512), (512, 256)):
            pg = pp.tile([P, 512], f32, space="PSUM", name="ps")
            for t, tsz in ((0, P), (1, 68)):
                nc.tensor.matmul(pg[0:mssz, 0:nsz],
                                 lhsT=PTf[0:tsz, t, ms0:ms0 + mssz],
                                 rhs=x_nc[0:tsz, t, n0:n0 + nsz],
                                 start=(t == 0), stop=(t == 1))
            nc.vector.tensor_copy(gt[0:mssz, n0:n0 + nsz], pg[0:mssz, 0:nsz])
        gath.append(gt)
    xo0, xo1, ntk = gath

    # feature-major views for the distance matmul via PE transposes (exact)
    xntT = pb.tile([P, 6, 196], f32, name="xntT")
    for ci in range(6):
        ptr = pp.tile([P, 196], f32, space="PSUM", name="ps")
        for (src_t, r0, rsz) in ((xo0, 0, P), (xo1, P, 10), (ntk, L, M)):
            nc.tensor.transpose(out=ptr[:, r0:r0 + rsz],
                                in_=src_t[0:rsz, ci * P:(ci + 1) * P],
                                identity=ident[0:rsz, 0:rsz])
        nc.vector.tensor_copy(xntT[:, ci, :], ptr[:])

    # column norms of gathered kept tokens -> 1/||xo_l|| (sqt reuses the PTf
    # slot: the attnG matmuls and the PT cast above are PTf's last readers)
    sqt = pb.tile([P, 6, L], f32, name="PTf")
    nc.vector.tensor_tensor(out=sqt[:], in0=xntT[:, :, 0:L], in1=xntT[:, :, 0:L],
                            op=OP.mult)
    pn = pp.tile([1, L], f32, space="PSUM", name="ps")
    for ci in range(6):
        nc.tensor.matmul(pn[:], lhsT=onesP[:, 0:1], rhs=sqt[:, ci, :],
                         start=(ci == 0), stop=(ci == 5))
    invxo = pb.tile([1, L], f32, name="invxo")
    dve_rsqrt(nc, pb, invxo[:], pn[:], [1, L], "nx")

    # raw distance [M, L] and scaled version for argmax
    pr = pp.tile([M, L], f32, space="PSUM", name="ps")
    for ci in range(6):
        nc.tensor.matmul(pr[:], lhsT=xntT[:, ci, L:196], rhs=xntT[:, ci, 0:L],
                         start=(ci == 0), stop=(ci == 5))
    praw = pb.tile([M, L], f32, name="praw")
    nc.vector.tensor_copy(praw[:], pr[:])
    pbc = pp.tile([M, L], f32, space="PSUM", name="ps")
    nc.tensor.matmul(pbc[:], lhsT=onesRow[0:1, 0:M], rhs=invxo[:],
                     start=True, stop=True)
    nc.vector.tensor_tensor(out=praw[:], in0=praw[:], in1=pbc[:], op=OP.mult)

    # argmax (first occurrence) -> one-hot firsthot [M, L]; praw holds the
    # scaled distances and is consumed in place
    rmax = pb.tile([M, 1], f32, name="rmax")
    nc.vector.tensor_reduce(out=rmax[:], in_=praw[:], axis=AX.X, op=OP.max)
    nc.vector.tensor_scalar(out=praw[:], in0=praw[:], scalar1=rmax[:, 0:1],
                            scalar2=None, op0=OP.is_equal)
    nc.vector.tensor_tensor(out=praw[:], in0=praw[:], in1=IotaLmB[0:M, :],
                            op=OP.mult)
    nc.vector.tensor_scalar(out=praw[:], in0=praw[:], scalar1=65536.0,
                            scalar2=None, op0=OP.add)
    mina = pb.tile([M, 1], f32, name="mina")
    nc.vector.tensor_reduce(out=mina[:], in_=praw[:], axis=AX.X, op=OP.min)
    fh = pb.tile([M, L], f32, name="fh")
    nc.vector.tensor_scalar(out=fh[:], in0=IotaF[0:M, 0:L], scalar1=mina[:, 0:1],
                            scalar2=None, op0=OP.is_equal)

    # weighted tokens (f32r: scatter feeds output values only, not selection)
    ntw = pb.tile([M, C], f32r, name="ntw")
    nc.vector.tensor_scalar(out=ntw[:], in0=ntk[0:M, :], scalar1=attnG[2][0:M, 0:1],
                            scalar2=None, op0=OP.mult)
    fhr = pb.tile([M, L], f32r, name="fhr")
    nc.vector.tensor_copy(fhr[:], fh[:])

    # scatter-add into kept rows + divide by merged attention
    for ki_, (ms0, mssz) in enumerate(((0, P), (P, 10))):
        pd = pp.tile([P, 1], f32, space="PSUM", name="ps")
        nc.tensor.matmul(pd[0:mssz, :], lhsT=fh[:, ms0:ms0 + mssz],
                         rhs=attnG[2][0:M, 0:1], start=True, stop=True)
        dsum = pb.tile([P, 1], f32, name=f"dsum{ki_}")
        nc.vector.tensor_tensor(out=dsum[0:mssz, :], in0=attnG[ki_][0:mssz, :],
                                in1=pd[0:mssz, :], op=OP.add)
        rd = pb.tile([P, 1], f32, name=f"rd{ki_}")
        newton_recip(nc, pb, rd[0:mssz, :], dsum[0:mssz, :], [mssz, 1], f"d{ki_}")
        xow = (xo0, xo1)[ki_]  # weighted in place; the gathered rows are dead
        nc.vector.tensor_scalar(out=xow[0:mssz, :], in0=xow[0:mssz, :],
                                scalar1=attnG[ki_][0:mssz, 0:1], scalar2=None,
                                op0=OP.mult)
        for (n0, nsz) in ((0, 512), (512, 256)):
            ps = pp.tile([P, 512], f32, space="PSUM", name="ps")
            nc.tensor.matmul(ps[0:mssz, 0:nsz], lhsT=fhr[:, ms0:ms0 + mssz],
                             rhs=ntw[:, n0:n0 + nsz], start=True, stop=True)
            nc.vector.tensor_tensor(out=xow[0:mssz, n0:n0 + nsz],
                                    in0=xow[0:mssz, n0:n0 + nsz],
                                    in1=ps[0:mssz, 0:nsz], op=OP.add)
        nc.vector.tensor_scalar(out=xow[0:mssz, :], in0=xow[0:mssz, :],
                                scalar1=rd[0:mssz, 0:1], scalar2=None, op0=OP.mult)
        nc.sync.dma_start(
            out=xsel_d[samp * NO + 1 + ms0: samp * NO + 1 + ms0 + mssz, :],
            in_=xow[0:mssz, :])


def _phase_c_chunk(nc, tc, pcs, pp, t0, tsz, xsel_d, fc1_r, fc2_r, ident,
                   out_flat, pc1):
    tiles = _ceil_tiles(tsz)
    nt = len(tiles)
    xc_sb = pcs.tile([P, 4, C], f32, name="xc_sb")
    xn2 = pc1.tile([P, 4, C], f32, name="xn2")
    for ti, (m0, msz) in enumerate(tiles):
        nc.sync.dma_start(out=xc_sb[0:msz, ti, :],
                          in_=xsel_d[t0 + m0: t0 + m0 + msz, :])
        layernorm_tile(nc, pcs, pp, xn2[0:msz, ti, :], xc_sb[0:msz, ti, :],
                       msz, False, "c")
    xnT = pc1.tile([P, 6, 512], bf16, name="xnT2")
    for ci in range(6):
        ptr = pp.tile([P, 512], f32, space="PSUM", name="ps")
        for ti, (m0, msz) in enumerate(tiles):
            nc.tensor.transpose(out=ptr[:, m0:m0 + msz],
                                in_=xn2[0:msz, ti, ci * P:(ci + 1) * P],
                                identity=ident[0:msz, 0:msz])
        nc.vector.tensor_copy(xnT[:, ci, 0:tsz], ptr[:, 0:tsz])

    # fc1 (bf16, resident) + gelu -> hT [H4, tsz] feature-major
    hT = pc1.tile([P, 24, 512], bf16, name="hT")
    for oi in range(24):
        pf = pp.tile([P, 512], f32, space="PSUM", name="ps")
        for ki in range(6):
            nc.tensor.matmul(pf[:, 0:tsz], lhsT=fc1_r[:, ki, oi * P:(oi + 1) * P],
                             rhs=xnT[:, ki, 0:tsz], start=(ki == 0), stop=(ki == 5))
        nc.scalar.activation(hT[:, oi, 0:tsz], pf[:, 0:tsz], AF.Gelu,
                             bias=KC["zero"][:, 0:1])

    # fc2 (bf16) + residual -> out
    for ti, (m0, msz) in enumerate(tiles):
        for (n0, nsz) in ((0, 512), (512, 256)):
            pf = pp.tile([P, 512], f32, space="PSUM", name="ps")
            for ki in range(24):
                nc.tensor.matmul(pf[0:msz, 0:nsz],
                                 lhsT=hT[:, ki, m0:m0 + msz],
                                 rhs=fc2_r[:, ki, n0:n0 + nsz],
                                 start=(ki == 0), stop=(ki == 23))
            nc.vector.tensor_tensor(out=xc_sb[0:msz, ti, n0:n0 + nsz],
                                    in0=xc_sb[0:msz, ti, n0:n0 + nsz],
                                    in1=pf[0:msz, 0:nsz], op=OP.add)
        nc.sync.dma_start(out=out_flat[t0 + m0: t0 + m0 + msz, :],
                          in_=xc_sb[0:msz, ti, :])


_NC_CACHE = None


def kernel(**inputs):
    global _NC_CACHE
    if _NC_CACHE is None:
        _NC_CACHE = build_nc()
    nc = _NC_CACHE

    import ml_dtypes
    x = np.ascontiguousarray(np.asarray(inputs["x"], dtype=np.float32))
    wnames = ["qkv_w", "proj_w", "fc1_w", "fc2_w"]
    ws = {k: np.ascontiguousarray(np.asarray(inputs[k], dtype=np.float32))
          for k in wnames}
    ws["fc1_w"] = ws["fc1_w"].astype(ml_dtypes.bfloat16)
    ws["fc2_w"] = ws["fc2_w"].astype(ml_dtypes.bfloat16)
    B = x.shape[0]
    n_cores = 8
    bl = B // n_cores
    in_maps = []
    for c in range(n_cores):
        m = {"x": x[c * bl:(c + 1) * bl]}
        m.update(ws)
        in_maps.append(m)
    res = run_bass_kernel_spmd(nc, in_maps, core_ids=list(range(n_cores)))
    out = np.concatenate([r["out"] for r in res.results], axis=0)
    return out.astype(np.float32)


# revision 46
# speedup vs baseline: 1.2633x; 1.2416x over previous
"""TRN2 Bass kernel for nn_Block_18227841204857 (EViT-style block with top-k token
merging). Data-parallel over batch: 8 cores x 16 samples.

Contract: kernel(**inputs) takes full unsharded inputs, returns full output
(128, 139, 768) float32.

Precision strategy: the top-k selection path (ln1, k-projection, cls-query q0,
cls scores, softmax-mean) stays true fp32; everything else off the selection
path runs f32r matmuls (1 cycle/row at free>=256 vs fp32's 4) or bf16 (MLP).
"""
import sys
sys.path.insert(0, "/opt/trn_rl_repo")

import math
import numpy as np

import concourse.bacc as bacc
import concourse.bass as bass
import concourse.mybir as mybir
from concourse.tile import TileContext
from concourse.masks import make_identity
from concourse.bass_utils import run_bass_kernel_spmd

P = 128
f32 = mybir.dt.float32
f32r = mybir.dt.float32r
bf16 = mybir.dt.bfloat16
i32 = mybir.dt.int32
AF = mybir.ActivationFunctionType
OP = mybir.AluOpType
AX = mybir.AxisListType

B_L = 16          # samples per core
N = 197           # tokens
C = 768           # channels
NH = 12           # heads
HD = 64           # head dim
L = 138           # kept tokens
M = 58            # pruned tokens
NO = 139          # output tokens (cls + kept)
H4 = 3072         # mlp hidden
EPS = 1e-5
NCHUNK = 8        # phase-A chunks (2 samples each)
T2 = 2 * N        # 394 tokens per chunk
QPAD = 256        # free size for f32r score matmuls (>=256 for 1 cycle/row)

LOG2E = float(np.float32(1.4426950408889634))
LN2 = float(np.float32(0.6931471805599453))
MAGIC = 12582912.0  # 1.5 * 2**23, round-to-nearest-int trick

KC = {}  # const tiles shared across build helpers


def _sample_tiles():
    # token tiles within one sample: (tile_idx, start, size)
    return [(0, 0, 128), (1, 128, 69)]


def _ceil_tiles(n):
    out = []
    s = 0
    while s < n:
        sz = min(P, n - s)
        out.append((s, sz))
        s += sz
    return out


def dve_exp(nc, sb, out_ap, in_ap, rows, cols,
            names=("exp_y", "exp_nf", "exp_t", "exp_p", "exp_ni")):
    """out = exp(in) elementwise, ~3e-7 rel accuracy, DVE+ACT only.

    exp(s) = 2^n * e^f,  n = round(s*log2e),  f = s - n*ln2  in [-0.347, 0.347]
    names lets callers alias the temporaries onto dead slots in their pool.
    """
    shp = [rows, cols]
    y = sb.tile(shp, f32, name=names[0])
    nf = sb.tile(shp, f32, name=names[1])
    t = sb.tile(shp, f32, name=names[2])
    p = sb.tile(shp, f32, name=names[3])
    u = y  # y is dead once nf is extracted; reuse as Horner ping-pong
    ni = sb.tile(shp, i32, name=names[4])

    nc.vector.tensor_scalar(out=y[:], in0=in_ap, scalar1=LOG2E, scalar2=MAGIC,
                            op0=OP.mult, op1=OP.add)
    nc.vector.tensor_scalar(out=nf[:], in0=y[:], scalar1=MAGIC, scalar2=None,
                            op0=OP.subtract)
    # t = s - nf*ln2   (scalar_tensor_tensor: (in0 op0 scalar) op1 in1)
    nc.vector.scalar_tensor_tensor(out=t[:], in0=nf[:], scalar=-LN2, in1=in_ap,
                                   op0=OP.mult, op1=OP.add)
    # Horner for e^t, t in [-0.35, 0.35]: coeffs 1/k!, k=8..0
    coef = [1.0 / math.factorial(k) for k in range(9)]
    nc.vector.tensor_scalar(out=p[:], in0=t[:], scalar1=coef[8], scalar2=coef[7],
                            op0=OP.mult, op1=OP.add)
    for k in range(6, -1, -1):
        nc.vector.tensor_tensor(out=u[:], in0=p[:], in1=t[:], op=OP.mult)
        nc.vector.tensor_scalar(out=p[:], in0=u[:], scalar1=coef[k], scalar2=None,
                                op0=OP.add)
    # 2^n via bit tricks: (n + 127) << 23, bitcast to f32
    nc.vector.tensor_copy(ni[:], nf[:])
    nc.vector.tensor_scalar(out=ni[:], in0=ni[:], scalar1=127, scalar2=None, op0=OP.add)
    nc.vector.tensor_scalar(out=ni[:], in0=ni[:], scalar1=23, scalar2=None,
                            op0=OP.arith_shift_left)
    nc.vector.tensor_tensor(out=out_ap, in0=p[:], in1=ni[:].bitcast(f32), op=OP.mult)


def newton_recip(nc, sb, out_ap, in_ap, shp, tag):
    """out = 1/in with one Newton refinement on DVE reciprocal."""
    r0 = sb.tile(shp, f32, name=f"nr_r0_{tag}")
    tt = sb.tile(shp, f32, name=f"nr_t_{tag}")
    nc.vector.reciprocal(r0[:], in_ap)
    nc.vector.tensor_tensor(out=tt[:], in0=r0[:], in1=in_ap, op=OP.mult)
    nc.vector.tensor_scalar(out=tt[:], in0=tt[:], scalar1=-1.0, scalar2=2.0,
                            op0=OP.mult, op1=OP.add)
    nc.vector.tensor_tensor(out=out_ap, in0=r0[:], in1=tt[:], op=OP.mult)


def dve_rsqrt(nc, sb, out_ap, in_ap, shp, tag):
    """out = 1/sqrt(in) on DVE only (quake seed + 3 Newton iters)."""
    v = sb.tile(shp, f32, name=f"rq_v_{tag}")
    yb = sb.tile(shp, i32, name=f"rq_yb_{tag}")
    t1 = sb.tile(shp, f32, name=f"rq_t1_{tag}")
    nc.vector.tensor_copy(v[:], in_ap)
    nc.vector.tensor_scalar(out=yb[:], in0=v[:].bitcast(i32), scalar1=1,
                            scalar2=None, op0=OP.arith_shift_right)
    nc.vector.tensor_scalar(out=yb[:], in0=yb[:], scalar1=-1,
                            scalar2=0x5F3759DF, op0=OP.mult, op1=OP.add)
    y = yb[:].bitcast(f32)
    for _ in range(3):
        nc.vector.tensor_tensor(out=t1[:], in0=v[:], in1=y, op=OP.mult)
        nc.vector.tensor_tensor(out=t1[:], in0=t1[:], in1=y, op=OP.mult)
        nc.vector.tensor_scalar(out=t1[:], in0=t1[:], scalar1=-0.5, scalar2=1.5,
                                op0=OP.mult, op1=OP.add)
        nc.vector.tensor_tensor(out=out_ap, in0=y, in1=t1[:], op=OP.mult)
        y = out_ap


def layernorm_tile(nc, sb, ps, out_ap, x_ap, rows, newton, tag,
                   xc_name=None, sq_name=None):
    """LayerNorm over free dim (768), w=1 b=0. out/x: [rows, C]."""
    shp = [rows, 1]
    rs = sb.tile(shp, f32, name=f"ln_rs_{tag}")
    nm = sb.tile(shp, f32, name=f"ln_nm_{tag}")
    xc = sb.tile([rows, C], f32, name=xc_name or f"ln_xc_{tag}")
    sq = sb.tile([rows, C], f32, name=sq_name or f"ln_sq_{tag}")
    vs = sb.tile(shp, f32, name=f"ln_vs_{tag}")
    sd = sb.tile(shp, f32, name=f"ln_sd_{tag}")
    nc.vector.tensor_reduce(out=rs[:], in_=x_ap, axis=AX.X, op=OP.add)
    nc.vector.tensor_scalar(out=nm[:], in0=rs[:], scalar1=-1.0 / C, scalar2=None,
                            op0=OP.mult)
    nc.vector.tensor_scalar(out=xc[:], in0=x_ap, scalar1=nm[:, 0:1], scalar2=None,
                            op0=OP.add)
    nc.vector.tensor_tensor(out=sq[:], in0=xc[:], in1=xc[:], op=OP.mult)
    nc.vector.tensor_reduce(out=vs[:], in_=sq[:], axis=AX.X, op=OP.add)
    nc.scalar.activation(sd[:], vs[:], AF.Sqrt, scale=1.0 / C,
                         bias=KC["eps"][0:rows, 0:1])
    inv = sb.tile(shp, f32, name=f"ln_inv_{tag}")
    if newton:
        newton_recip(nc, sb, inv[:], sd[:], shp, f"ln_{tag}")
    else:
        nc.vector.reciprocal(inv[:], sd[:])
    nc.vector.tensor_scalar(out=out_ap, in0=xc[:], scalar1=inv[:, 0:1], scalar2=None,
                            op0=OP.mult)


def build_nc():
    nc = bacc.Bacc("TRN2", target_bir_lowering=False, debug=False, num_swdge_queues=1)

    x_in = nc.declare_dram_parameter("x", [B_L, N, C], f32, isOutput=False)
    qkv_w = nc.declare_dram_parameter("qkv_w", [C, 3 * C], f32, isOutput=False)
    proj_w = nc.declare_dram_parameter("proj_w", [C, C], f32, isOutput=False)
    fc1_w = nc.declare_dram_parameter("fc1_w", [C, H4], bf16, isOutput=False)
    fc2_w = nc.declare_dram_parameter("fc2_w", [H4, C], bf16, isOutput=False)
    out_ext = nc.declare_dram_parameter("out", [B_L, NO, C], f32, isOutput=True)

    out_flat = out_ext.ap().rearrange("b n c -> (b n) c")

    with TileContext(nc) as tc:
        _build_body(nc, tc, x_in, qkv_w, proj_w, fc1_w, fc2_w, out_flat)
    nc.finalize()
    return nc


def _build_body(nc, tc, x_in, qkv_w, proj_w, fc1_w, fc2_w, out_flat):
    from contextlib import ExitStack

    ctx = ExitStack()
    with ctx:
        # ---------- constants ----------
        pc = ctx.enter_context(tc.tile_pool(name="const", bufs=1))
        pdram = ctx.enter_context(tc.tile_pool(name="dram", bufs=1, space="DRAM"))

        ident = pc.tile([P, P], f32)
        make_identity(nc, ident[:])
        onesP = pc.tile([P, 1], f32)
        nc.vector.memset(onesP[:], 1.0)
        onesRow = pc.tile([1, P], f32)
        nc.vector.memset(onesRow[:], 1.0)
        epsb = pc.tile([P, 1], f32)
        nc.vector.memset(epsb[:], EPS)
        zerob = pc.tile([P, 1], f32)
        nc.vector.memset(zerob[:], 0.0)
        KC["eps"] = epsb
        KC["zero"] = zerob

        iota_i = pc.tile([P, 1], i32)
        nc.gpsimd.iota(iota_i[:], pattern=[[1, 1]], base=0, channel_multiplier=1)
        iotaPf = pc.tile([P, 1], f32)
        nc.vector.tensor_copy(iotaPf[:], iota_i[:])
        iotaPf128 = pc.tile([P, 1], f32)
        nc.vector.tensor_scalar(out=iotaPf128[:], in0=iotaPf[:], scalar1=128.0,
                                scalar2=None, op0=OP.add)

        iota_f_i = pc.tile([P, 196], i32)
        nc.gpsimd.iota(iota_f_i[:], pattern=[[1, 196]], base=0, channel_multiplier=0)
        IotaF = pc.tile([P, 196], f32)
        nc.vector.tensor_copy(IotaF[:], iota_f_i[:])
        IotaLmB = pc.tile([P, L], f32)
        nc.vector.tensor_scalar(out=IotaLmB[:], in0=IotaF[:, 0:L], scalar1=-65536.0,
                                scalar2=None, op0=OP.add)

        LT0 = pc.tile([P, 196], f32)
        nc.vector.tensor_scalar(out=LT0[:], in0=IotaF[:], scalar1=iotaPf[:, 0:1],
                                scalar2=None, op0=OP.is_lt)
        LT1 = pc.tile([P, 196], f32)
        nc.vector.tensor_scalar(out=LT1[:], in0=IotaF[:], scalar1=iotaPf128[:, 0:1],
                                scalar2=None, op0=OP.is_lt)
        TRI0 = pc.tile([P, 196], f32)
        nc.vector.tensor_scalar(out=TRI0[:], in0=IotaF[:], scalar1=iotaPf[:, 0:1],
                                scalar2=None, op0=OP.is_ge)
        TRI1 = pc.tile([P, 196], f32)
        nc.vector.tensor_scalar(out=TRI1[:], in0=IotaF[:], scalar1=iotaPf128[:, 0:1],
                                scalar2=None, op0=OP.is_ge)

        # ---------- DRAM scratch ----------
        xattn_d = pdram.tile([B_L * N, C], f32)
        xsel_d = pdram.tile([B_L * NO, C], f32)
        clsD = pdram.tile([B_L, 196], f32)

        # ================= PHASE A + interleaved selection =================
        with ExitStack() as actx:
            pw = actx.enter_context(tc.tile_pool(name="aw", bufs=1))
            pp = actx.enter_context(tc.tile_pool(name="aps", bufs=6, space="PSUM"))

            # resident weights: all f32r; the fp32 selection path uses the
            # precomputed u-vectors (u_h = Wk_h @ q0_h) instead of fp32 k
            qkvKr = pw.tile([P, 6, C], f32r)
            qkvQr = pw.tile([P, 6, C], f32r)
            qkvVr = pw.tile([P, 6, C], f32r)
            projr = pw.tile([P, 6, C], f32r)
            u_all = pw.tile([P, 6, B_L * NH], f32)

            with ExitStack() as sctx:
                pstage = sctx.enter_context(tc.tile_pool(name="stg", bufs=1))
                # separate staging tiles so the three weight loads + casts all
                # run in parallel (across DMA and the three copy engines)
                wq = pstage.tile([P, 6, C], f32, name="wq")
                wv = pstage.tile([P, 6, C], f32, name="wv")
                wp = pstage.tile([P, 6, C], f32, name="wp")
                wk = pstage.tile([P, 6, C], f32, name="wk")
                nc.sync.dma_start(
                    out=wq[:],
                    in_=qkv_w.ap()[:, 0:C].rearrange("(kt p) o -> p kt o", p=P))
                nc.sync.dma_start(
                    out=wv[:],
                    in_=qkv_w.ap()[:, 2 * C:3 * C].rearrange("(kt p) o -> p kt o", p=P))
                nc.sync.dma_start(
                    out=wp[:],
                    in_=proj_w.ap().rearrange("(kt p) o -> p kt o", p=P))
                nc.sync.dma_start(
                    out=wk[:],
                    in_=qkv_w.ap()[:, C:2 * C].rearrange("(kt p) o -> p kt o", p=P))
                nc.scalar.copy(qkvVr[:], wv[:])
                nc.vector.tensor_copy(projr[:], wp[:])
                nc.scalar.copy(qkvKr[:], wk[:])
                # Wk^T (fp32, exact transposes) for the u-vector pre-pass
                wkT = pstage.tile([P, 6, C], f32, name="wkT")
                for bt in range(6):
                    ptw = pp.tile([P, P], f32, space="PSUM", name="ps")
                    for at in range(6):
                        nc.tensor.transpose(
                            out=ptw[:, 0:P],
                            in_=wk[:, at, bt * P:(bt + 1) * P],
                            identity=ident[:])
                        nc.vector.tensor_copy(wkT[:, bt, at * P:(at + 1) * P],
                                              ptw[:, 0:P])
                # --- q0 pre-pass: fp32 cls-query for all 16 samples ---
                xcls = pstage.tile([B_L, C], f32, name="xcls")
                nc.sync.dma_start(out=xcls[:], in_=x_in.ap()[:, 0, :])
                xn0 = pstage.tile([B_L, C], f32, name="xn0")
                layernorm_tile(nc, pstage, pp, xn0[:], xcls[:], B_L, True, "q0")
                xn0T = pstage.tile([P, 6, B_L], f32, name="xn0T")
                for ci in range(6):
                    ptq = pp.tile([P, B_L], f32, space="PSUM", name="ps")
                    nc.tensor.transpose(out=ptq[:, 0:B_L],
                                        in_=xn0[0:B_L, ci * P:(ci + 1) * P],
                                        identity=ident[0:B_L, 0:B_L])
                    nc.vector.tensor_copy(xn0T[:, ci, :], ptq[:])
                q0_all = pstage.tile([B_L, C], f32, name="q0_all")
                for (n0, nsz) in ((0, 512), (512, 256)):
                    psq = pp.tile([B_L, 512], f32, space="PSUM", name="ps")
                    for ki in range(6):
                        nc.tensor.matmul(psq[0:B_L, 0:nsz],
                                         lhsT=xn0T[:, ki, :],
                                         rhs=wq[:, ki, n0:n0 + nsz],
                                         start=(ki == 0), stop=(ki == 5))
                    nc.vector.tensor_copy(q0_all[:, n0:n0 + nsz], psq[0:B_L, 0:nsz])
                q0T_all = pstage.tile([P, 6, B_L], f32, name="q0T_all")
                for ci in range(6):
                    ptq = pp.tile([P, B_L], f32, space="PSUM", name="ps")
                    nc.tensor.transpose(out=ptq[:, 0:B_L],
                                        in_=q0_all[0:B_L, ci * P:(ci + 1) * P],
                                        identity=ident[0:B_L, 0:B_L])
                    nc.vector.tensor_copy(q0T_all[:, ci, :], ptq[:])
                nc.vector.tensor_copy(qkvQr[:], wq[:])
                # u[c, (samp, h)] = sum_d Wk[c, h*64+d] * q0[samp, h*64+d]
                # (fp32; replaces the fp32 k-projection in the score path)
                for h in range(NH):
                    bt, off = h // 2, (h % 2) * HD
                    for ft in range(6):
                        psu = pp.tile([P, B_L], f32, space="PSUM", name="ps")
                        nc.tensor.matmul(
                            psu[:],
                            lhsT=wkT[off:off + HD, bt, ft * P:(ft + 1) * P],
                            rhs=q0T_all[off:off + HD, bt, :],
                            start=True, stop=True)
                        dst = u_all[:, ft, :].rearrange(
                            "p (s h) -> p s h", h=NH)[:, :, h]
                        nc.vector.tensor_copy(dst, psu[:])

            # main working pools enter AFTER the staging scope exits so its
            # SBUF is reclaimed (stack LIFO)
            pa = actx.enter_context(tc.tile_pool(name="aa", bufs=1))
            pa2 = actx.enter_context(tc.tile_pool(name="aa2", bufs=1))
            pb2 = actx.enter_context(tc.tile_pool(name="b2i", bufs=1))

            # software pipeline: phase-B work for chunk ch-1 is split into
            # stages interleaved between chunk ch's PE-bound phase-A sections,
            # so the in-order PE queue never blocks long on phase-B's
            # vector-engine chains.
            consts = dict(LT0=LT0, LT1=LT1, TRI0=TRI0, TRI1=TRI1, IotaF=IotaF,
                          IotaLmB=IotaLmB, onesP=onesP, onesRow=onesRow,
                          ident=ident)
            noop9 = [lambda: None] * 9
            prev = None
            for ch in range(NCHUNK):
                cur = make_rest_stages(nc, pb2, pp, ch, xattn_d, xsel_d,
                                       consts)
                _phase_a_chunk(nc, tc, pa, pa2, pp, ch, x_in, qkvKr,
                               qkvQr, qkvVr, projr, ident, onesP,
                               xattn_d, xsel_d,
                               prev[1:] if prev else noop9,
                               cur[0],
                               lambda xnT, _ch=ch: _phase_b_score(
                                   nc, pb2, pp, _ch, u_all, xnT))
                prev = cur
            for st_fn in prev[1:]:
                st_fn()

        # ================= PHASE C: MLP =================
        with ExitStack() as cctx:
            pw = cctx.enter_context(tc.tile_pool(name="cw", bufs=1))
            pcs = cctx.enter_context(tc.tile_pool(name="cc", bufs=2))
            pc1 = cctx.enter_context(tc.tile_pool(name="cc1", bufs=1))
            pp = cctx.enter_context(tc.tile_pool(name="cps", bufs=6, space="PSUM"))

            fc2_r = pw.tile([P, 24, C], bf16)
            w2view = fc2_w.ap().rearrange("(kt p) o -> p kt o", p=P)
            nc.sync.dma_start(out=fc2_r[:], in_=w2view[:])
            fc1_r = pw.tile([P, 6, H4], bf16)
            nc.sync.dma_start(out=fc1_r[:],
                              in_=fc1_w.ap().rearrange("(kt p) o -> p kt o", p=P))

            TOK = B_L * NO  # 2224
            TCK = 512
            nch = (TOK + TCK - 1) // TCK
            for ci in range(nch):
                t0 = ci * TCK
                tsz = min(TCK, TOK - t0)
                _phase_c_chunk(nc, tc, pcs, pp, t0, tsz, xsel_d, fc1_r,
                               fc2_r, ident, out_flat, pc1)


def _phase_b_score(nc, pb, pp, ch, u_all, xnT):
    """Raw cls scores for samples 2ch, 2ch+1: s[h, j] = xn_j . u_h (fp32),
    mathematically q0_h . k_{j,h} with the Wk contraction pre-folded into u."""
    s_all = pb.tile([NH, 2, N], f32, name="s_all")
    for s2 in range(2):
        samp = 2 * ch + s2
        psc = pp.tile([NH, N], f32, space="PSUM", name="ps")
        for ki in range(6):
            nc.tensor.matmul(psc[:],
                             lhsT=u_all[:, ki, samp * NH:(samp + 1) * NH],
                             rhs=xnT[:, ki, s2 * N:(s2 + 1) * N],
                             start=(ki == 0), stop=(ki == 5))
        nc.vector.tensor_scalar(out=s_all[:, s2, :], in0=psc[:],
                                scalar1=0.125, scalar2=None, op0=OP.mult)


def make_rest_stages(nc, pb, pp, ch, xattn_d, xsel_d, consts):
    """Phase-B (softmax/topk/merge) for chunk ch, split into 10 emit-stages.

    The caller interleaves these between phase-A sections of the next chunk.
    Each stage's cross-engine dependencies were produced at least one stage
    earlier, so no engine queue-head blocks for long. Sample s2=0 runs one
    stage ahead of s2=1 through the gather/distance/scatter pipeline.
    """
    LT0, LT1 = consts["LT0"], consts["LT1"]
    TRI0, TRI1 = consts["TRI0"], consts["TRI1"]
    IotaF, IotaLmB = consts["IotaF"], consts["IotaLmB"]
    onesP, onesRow, ident = consts["onesP"], consts["onesRow"], consts["ident"]
    S = {}
    mslices = ((0, P), (P, 10), (L, M))   # keep0, keep1, compl

    def r0_softmax():
        s_all = pb.tile([NH, 2, N], f32, name="s_all")
        smax = pb.tile([NH, 2], f32, name="smax")
        nc.vector.tensor_reduce(out=smax[:], in_=s_all[:], axis=AX.X, op=OP.max)
        sbc = bass.AP(tensor=smax[:].tensor, offset=smax[:].offset,
                      ap=[[smax[:].ap[0][0], NH], [1, 2], [0, N]])
        nc.vector.tensor_tensor(out=s_all[:], in0=s_all[:], in1=sbc,
                                op=OP.subtract)
        nc.vector.tensor_scalar(out=s_all[:], in0=s_all[:], scalar1=-80.0,
                                scalar2=None, op0=OP.max)
        e_all = pb.tile([NH, 2, N], f32, name="e_all")
        dve_exp(nc, pb, e_all[:].rearrange("h s n -> h (s n)"),
                s_all[:].rearrange("h s n -> h (s n)"), NH, 2 * N,
                names=("gath0", "gath1", "gath2", "sqt", "xntT"))
        den = pb.tile([NH, 2], f32, name="den")
        nc.vector.tensor_reduce(out=den[:], in_=e_all[:], axis=AX.X, op=OP.add)
        rden = smax
        newton_recip(nc, pb, rden[:], den[:], [NH, 2], "den")
        rbc = bass.AP(tensor=rden[:].tensor, offset=rden[:].offset,
                      ap=[[rden[:].ap[0][0], NH], [1, 2], [0, N]])
        nc.vector.tensor_tensor(out=e_all[:], in0=e_all[:], in1=rbc, op=OP.mult)
        S["a_all"] = e_all

    def r1_clsmean():
        a_all = S["a_all"]
        cls_all = pb.tile([1, 2, 196], f32, name="cls_all")
        for s2 in range(2):
            pcm = pp.tile([1, 196], f32, space="PSUM", name="ps")
            nc.tensor.matmul(pcm[:], lhsT=onesP[0:NH, 0:1],
                             rhs=a_all[:, s2, 1:N], start=True, stop=True)
            nc.vector.tensor_scalar(out=cls_all[0:1, s2, :], in0=pcm[:],
                                    scalar1=1.0 / 12.0, scalar2=None,
                                    op0=OP.mult)
        clsPT = pb.tile([P, 2, 2], f32, name="clsPT")
        for s2 in range(2):
            ptt = pp.tile([P, 1], f32, space="PSUM", name="ps")
            nc.tensor.transpose(out=ptt[:, 0:1], in_=cls_all[0:1, s2, 0:P],
                                identity=ident[0:1, 0:1])
            nc.vector.tensor_copy(clsPT[:, 0, s2:s2 + 1], ptt[:, 0:1])
            ptt2 = pp.tile([P, 1], f32, space="PSUM", name="ps")
            nc.tensor.transpose(out=ptt2[0:68, 0:1], in_=cls_all[0:1, s2, P:196],
                                identity=ident[0:1, 0:1])
            nc.vector.tensor_copy(clsPT[0:68, 1, s2:s2 + 1], ptt2[0:68, 0:1])
        S["cls_all"], S["clsPT"] = cls_all, clsPT

    def r2_vf():
        cls_all = S["cls_all"]
        vfp = []
        for s2 in range(2):
            VFp = pp.tile([P, 196], f32, space="PSUM", name="ps")
            nc.tensor.matmul(VFp[:], lhsT=onesRow[0:1, 0:P],
                             rhs=cls_all[0:1, s2, :], start=True, stop=True)
            vfp.append(VFp)
        # VF reuses cls_all's slot (both broadcast matmuls above are its last
        # readers)
        VF = pb.tile([P, 2, 196], f32, name="cls_all")
        for s2 in range(2):
            nc.scalar.copy(VF[:, s2, :], vfp[s2][:])
        S["VF"] = VF

    def r3_ranks():
        VF, clsPT = S["VF"], S["clsPT"]
        rank_all = pb.tile([P, 2, 2], f32, name="rank_all")
        pruned_all = pb.tile([P, 2, 2], f32, name="pruned_all")
        for s2 in range(2):
            for t, lt in ((0, LT0), (1, LT1)):
                srcv = clsPT[:, t, s2:s2 + 1]
                vp = bass.AP(tensor=srcv.tensor, offset=srcv.offset,
                             ap=[srcv.ap[0], [0, 196]])
                gt = pb.tile([P, 196], f32, name="rk_gt")
                eq = pb.tile([P, 196], f32, name="praw")
                nc.vector.tensor_tensor(out=gt[:], in0=VF[:, s2, :], in1=vp,
                                        op=OP.is_gt)
                nc.vector.tensor_tensor(out=eq[:], in0=VF[:, s2, :], in1=vp,
                                        op=OP.is_equal)
                nc.vector.tensor_tensor(out=eq[:], in0=eq[:], in1=lt[:],
                                        op=OP.mult)
                nc.vector.tensor_tensor(out=gt[:], in0=gt[:], in1=eq[:],
                                        op=OP.add)
                nc.vector.tensor_reduce(out=rank_all[:, t, s2:s2 + 1],
                                        in_=gt[:], axis=AX.X, op=OP.add)
                nc.vector.tensor_scalar(out=pruned_all[:, t, s2:s2 + 1],
                                        in0=rank_all[:, t, s2:s2 + 1],
                                        scalar1=137.5, scalar2=None,
                                        op0=OP.is_gt)
        S["rank_all"], S["pruned_all"] = rank_all, pruned_all

    def _build_ptf(s2):
        rank_all, pruned_all, posP = S["rank_all"], S["pruned_all"], S["posP"]
        PTf = pb.tile([P, 2, 196], f32, name="PTf")
        for t, tsz in ((0, P), (1, 68)):
            nc.vector.tensor_scalar(out=PTf[0:tsz, t, 0:L],
                                    in0=IotaF[0:tsz, 0:L],
                                    scalar1=rank_all[0:tsz, t, s2:s2 + 1],
                                    scalar2=None, op0=OP.is_equal)
            nc.vector.tensor_scalar(out=PTf[0:tsz, t, L:196],
                                    in0=IotaF[0:tsz, 1:59],
                                    scalar1=posP[0:tsz, t, s2:s2 + 1],
                                    scalar2=None, op0=OP.is_equal)
            nc.vector.tensor_scalar(out=PTf[0:tsz, t, L:196],
                                    in0=PTf[0:tsz, t, L:196],
                                    scalar1=pruned_all[0:tsz, t, s2:s2 + 1],
                                    scalar2=None, op0=OP.mult)
        S[f"PTf{s2}"] = PTf

    def r4_cumsum():
        pruned_all = S["pruned_all"]
        posP = pb.tile([P, 2, 2], f32, name="posP")
        pp0 = pp.tile([P, 2], f32, space="PSUM", name="ps")
        nc.tensor.matmul(pp0[:], lhsT=TRI0[:, 0:P], rhs=pruned_all[:, 0, :],
                         start=True, stop=False)
        nc.tensor.matmul(pp0[:], lhsT=TRI1[0:68, 0:P],
                         rhs=pruned_all[0:68, 1, :], start=False, stop=True)
        nc.vector.tensor_copy(posP[:, 0, :], pp0[:])
        pp1 = pp.tile([P, 2], f32, space="PSUM", name="ps")
        nc.tensor.matmul(pp1[0:68, :], lhsT=TRI0[:, P:196],
                         rhs=pruned_all[:, 0, :], start=True, stop=False)
        nc.tensor.matmul(pp1[0:68, :], lhsT=TRI1[0:68, P:196],
                         rhs=pruned_all[0:68, 1, :], start=False, stop=True)
        nc.vector.tensor_copy(posP[0:68, 1, :], pp1[0:68, :])
        S["posP"] = posP
        _build_ptf(0)

    def _gathers(s2):
        samp = 2 * ch + s2
        PTf, clsPT = S[f"PTf{s2}"], S["clsPT"]
        if s2 == 0:
            # one tile per rest-instance, shared by both samples via disjoint
            # columns (subtile deps keep the two samples' accesses independent;
            # a second pb.tile() call would rotate the slot and deadlock
            # against sample 0's later-stage readers)
            S["attnG"] = [pb.tile([P, 2], f32, name=f"attn_g{ms0}")
                          for (ms0, _) in mslices]
            S["x_nc"] = pb.tile([P, 2, C], f32, name="e_all")
            S["gath"] = [pb.tile([P, 2, C], f32, name=f"gath{gi}")
                         for gi in range(3)]
        attnG = S["attnG"]
        for gi, (ms0, mssz) in enumerate(mslices):
            pg = pp.tile([P, 1], f32, space="PSUM", name="ps")
            for t, tsz in ((0, P), (1, 68)):
                nc.tensor.matmul(pg[0:mssz, :],
                                 lhsT=PTf[0:tsz, t, ms0:ms0 + mssz],
                                 rhs=clsPT[0:tsz, t, s2:s2 + 1],
                                 start=(t == 0), stop=(t == 1))
            nc.vector.tensor_copy(attnG[gi][0:mssz, s2:s2 + 1], pg[0:mssz, :])
        x_nc = S["x_nc"]
        nc.sync.dma_start(out=x_nc[:, 0, :],
                          in_=xattn_d[samp * N + 1: samp * N + 129, :])
        nc.sync.dma_start(out=x_nc[0:68, 1, :],
                          in_=xattn_d[samp * N + 129: samp * N + 197, :])
        for gi, (ms0, mssz) in enumerate(mslices):
            gt = S["gath"][gi]
            for (n0, nsz) in ((0, 512), (512, 256)):
                pg = pp.tile([P, 512], f32, space="PSUM", name="ps")
                for t, tsz in ((0, P), (1, 68)):
                    nc.tensor.matmul(pg[0:mssz, 0:nsz],
                                     lhsT=PTf[0:tsz, t, ms0:ms0 + mssz],
                                     rhs=x_nc[0:tsz, t, n0:n0 + nsz],
                                     start=(t == 0), stop=(t == 1))
                nc.vector.tensor_copy(gt[0:mssz, s2, n0:n0 + nsz],
                                      pg[0:mssz, 0:nsz])

    def _norms(s2):
        xo0, xo1, ntk = S["gath"]
        xntT = pb.tile([P, 6, 196], f32, name="xntT")
        for ci in range(6):
            ptr = pp.tile([P, 196], f32, space="PSUM", name="ps")
            for (src_t, r0, rsz) in ((xo0, 0, P), (xo1, P, 10), (ntk, L, M)):
                nc.tensor.transpose(out=ptr[:, r0:r0 + rsz],
                                    in_=src_t[0:rsz, s2, ci * P:(ci + 1) * P],
                                    identity=ident[0:rsz, 0:rsz])
            nc.vector.tensor_copy(xntT[:, ci, :], ptr[:])
        sqt = pb.tile([P, 6, L], f32, name="sqt")
        nc.vector.tensor_tensor(out=sqt[:], in0=xntT[:, :, 0:L],
                                in1=xntT[:, :, 0:L], op=OP.mult)
        pn = pp.tile([1, L], f32, space="PSUM", name="ps")
        for ci in range(6):
            nc.tensor.matmul(pn[:], lhsT=onesP[:, 0:1], rhs=sqt[:, ci, :],
                             start=(ci == 0), stop=(ci == 5))
        invxo = pb.tile([1, L], f32, name="rank_all")
        dve_rsqrt(nc, pb, invxo[:], pn[:], [1, L], "nx")
        S[f"xntT{s2}"], S[f"invxo{s2}"] = xntT, invxo

    def _dist(s2):
        xntT, invxo = S[f"xntT{s2}"], S[f"invxo{s2}"]
        attnG = S["attnG"]
        ntk = S["gath"][2]
        pr = pp.tile([M, L], f32, space="PSUM", name="ps")
        for ci in range(6):
            nc.tensor.matmul(pr[:], lhsT=xntT[:, ci, L:196],
                             rhs=xntT[:, ci, 0:L],
                             start=(ci == 0), stop=(ci == 5))
        praw = pb.tile([M, L], f32, name="praw")
        nc.vector.tensor_copy(praw[:], pr[:])
        pbc = pp.tile([M, L], f32, space="PSUM", name="ps")
        nc.tensor.matmul(pbc[:], lhsT=onesRow[0:1, 0:M], rhs=invxo[:],
                         start=True, stop=True)
        nc.vector.tensor_tensor(out=praw[:], in0=praw[:], in1=pbc[:],
                                op=OP.mult)
        rmax = pb.tile([M, 1], f32, name="rmax")
        nc.vector.tensor_reduce(out=rmax[:], in_=praw[:], axis=AX.X, op=OP.max)
        nc.vector.tensor_scalar(out=praw[:], in0=praw[:],
                                scalar1=rmax[:, 0:1], scalar2=None,
                                op0=OP.is_equal)
        nc.vector.tensor_tensor(out=praw[:], in0=praw[:], in1=IotaLmB[0:M, :],
                                op=OP.mult)
        nc.vector.tensor_scalar(out=praw[:], in0=praw[:], scalar1=65536.0,
                                scalar2=None, op0=OP.add)
        mina = pb.tile([M, 1], f32, name="mina")
        nc.vector.tensor_reduce(out=mina[:], in_=praw[:], axis=AX.X, op=OP.min)
        fh = pb.tile([M, L], f32, name="cls_all")
        nc.vector.tensor_scalar(out=fh[:], in0=IotaF[0:M, 0:L],
                                scalar1=mina[:, 0:1], scalar2=None,
                                op0=OP.is_equal)
        ntw = pb.tile([M, C], f32r, name="rk_gt")
        nc.vector.tensor_scalar(out=ntw[:], in0=ntk[0:M, s2, :],
                                scalar1=attnG[2][0:M, s2:s2 + 1],
                                scalar2=None, op0=OP.mult)
        fhr = pb.tile([M, L], f32r, name="clsPT")
        nc.vector.tensor_copy(fhr[:], fh[:])
        S[f"fh{s2}"], S[f"fhr{s2}"], S[f"ntw{s2}"] = fh, fhr, ntw

    def _scatter(s2):
        samp = 2 * ch + s2
        fh, fhr, ntw = S[f"fh{s2}"], S[f"fhr{s2}"], S[f"ntw{s2}"]
        attnG = S["attnG"]
        gath = S["gath"]
        for ki_, (ms0, mssz) in enumerate(((0, P), (P, 10))):
            pd = pp.tile([P, 1], f32, space="PSUM", name="ps")
            nc.tensor.matmul(pd[0:mssz, :], lhsT=fh[:, ms0:ms0 + mssz],
                             rhs=attnG[2][0:M, s2:s2 + 1], start=True,
                             stop=True)
            dsum = pb.tile([P, 1], f32, name=f"dsum{ki_}")
            nc.vector.tensor_tensor(out=dsum[0:mssz, :],
                                    in0=attnG[ki_][0:mssz, s2:s2 + 1],
                                    in1=pd[0:mssz, :], op=OP.add)
            rd = pb.tile([P, 1], f32, name=f"rd{ki_}")
            newton_recip(nc, pb, rd[0:mssz, :], dsum[0:mssz, :], [mssz, 1],
                         f"d{ki_}")
            xow = gath[ki_]
            nc.vector.tensor_scalar(out=xow[0:mssz, s2, :],
                                    in0=xow[0:mssz, s2, :],
                                    scalar1=attnG[ki_][0:mssz, s2:s2 + 1],
                                    scalar2=None, op0=OP.mult)
            for (n0, nsz) in ((0, 512), (512, 256)):
                ps = pp.tile([P, 512], f32, space="PSUM", name="ps")
                nc.tensor.matmul(ps[0:mssz, 0:nsz], lhsT=fhr[:, ms0:ms0 + mssz],
                                 rhs=ntw[:, n0:n0 + nsz], start=True, stop=True)
                nc.vector.tensor_tensor(out=xow[0:mssz, s2, n0:n0 + nsz],
                                        in0=xow[0:mssz, s2, n0:n0 + nsz],
                                        in1=ps[0:mssz, 0:nsz], op=OP.add)
            nc.vector.tensor_scalar(out=xow[0:mssz, s2, :],
                                    in0=xow[0:mssz, s2, :],
                                    scalar1=rd[0:mssz, 0:1], scalar2=None,
                                    op0=OP.mult)
            nc.sync.dma_start(
                out=xsel_d[samp * NO + 1 + ms0: samp * NO + 1 + ms0 + mssz, :],
                in_=xow[0:mssz, s2, :])

    def r5():
        _gathers(0)
        _build_ptf(1)

    def r6():
        _norms(0)
        _gathers(1)

    def r7():
        _dist(0)
        _norms(1)

    def r8():
        _scatter(0)
        _dist(1)

    def r9():
        _scatter(1)

    return [r0_softmax, r1_clsmean, r2_vf, r3_ranks, r4_cumsum, r5, r6, r7, r9
            ] if False else [r0_softmax, r1_clsmean, r2_vf, r3_ranks,
                             r4_cumsum, r5, r6, r7, r8, r9]


def _phase_a_chunk(nc, tc, pa, pa2, pp, ch, x_in, qkvKr, qkvQr, qkvVr, projr,
                   ident, onesP, xattn_d, xsel_d):
    st = _sample_tiles()
    x_sb = pa2.tile([P, 2, 2, C], f32, name="x_sb")
    xn_sb = pa.tile([P, 2, 2, C], f32, name="xn_sb")
    for s2 in range(2):
        samp = 2 * ch + s2
        for (mt, m0, msz) in st:
            nc.sync.dma_start(out=x_sb[0:msz, s2, mt, :],
                              in_=x_in.ap()[samp, m0:m0 + msz, :])
            layernorm_tile(nc, pa, pp, xn_sb[0:msz, s2, mt, :],
                           x_sb[0:msz, s2, mt, :], msz, True, "a",
                           sq_name="v_blk")

    # transpose ln1 out -> feature-major [C, T2]: fp32 + f32r twin copies
    xnT = pa.tile([P, 6, T2], f32, name="xnT")
    xnTr = pa.tile([P, 6, T2], f32r, name="xnTr")
    for ci in range(6):
        ptr = pp.tile([P, T2], f32, space="PSUM", name="ps")
        for s2 in range(2):
            for (mt, m0, msz) in st:
                nc.tensor.transpose(
                    out=ptr[:, s2 * N + m0: s2 * N + m0 + msz],
                    in_=xn_sb[0:msz, s2, mt, ci * P:(ci + 1) * P],
                    identity=ident[0:msz, 0:msz])
        nc.vector.tensor_copy(xnT[:, ci, :], ptr[:])
        nc.scalar.copy(xnTr[:, ci, :], ptr[:])

    # q feature-major f32r
    qTr = pa.tile([P, 6, T2], f32r, name="qTr")
    for oi in range(6):
        pq = pp.tile([P, T2], f32, space="PSUM", name="ps")
        for ki in range(6):
            nc.tensor.matmul(
                pq[:],
                lhsT=qkvQr[:, ki, oi * P:(oi + 1) * P],
                rhs=xnTr[:, ki, :], start=(ki == 0), stop=(ki == 5))
        nc.vector.tensor_copy(qTr[:, oi, :], pq[:])

    # v token-major with leading ones column per head: [tok, (12, 65)]
    v_blk = pa.tile([P, 2, 2, NH, 65], f32, name="v_blk")
    nc.vector.memset(v_blk[:, :, :, :, 0:1], 1.0)
    for s2 in range(2):
        for (mt, m0, msz) in st:
            for nc_i, (n0, nsz) in enumerate(((0, 512), (512, 256))):
                pv = pp.tile([P, 512], f32, space="PSUM", name="ps")
                for ki in range(6):
                    nc.tensor.matmul(
                        pv[0:msz, 0:nsz],
                        lhsT=xnTr[:, ki, s2 * N + m0: s2 * N + m0 + msz],
                        rhs=qkvVr[:, ki, n0:n0 + nsz],
                        start=(ki == 0), stop=(ki == 5))
                h0 = n0 // HD
                nhh = nsz // HD
                nc.vector.tensor_copy(v_blk[0:msz, s2, mt, h0:h0 + nhh, 1:65],
                                      pv[0:msz, 0:nsz].rearrange(
                                          "p (h d) -> p h d", d=HD))

    # k feature-major f32r (the fp32 selection path no longer needs k — the
    # scores use the u-vectors against fp32 xnT). kTr reuses xnTr's slot
    # (the v section above is xnTr's last reader).
    kTr = pa.tile([P, 6, T2], f32r, name="xnTr")
    for oi in range(6):
        pq = pp.tile([P, T2], f32, space="PSUM", name="ps")
        for ki in range(6):
            nc.tensor.matmul(
                pq[:],
                lhsT=qkvKr[:, ki, oi * P:(oi + 1) * P],
                rhs=xnTr[:, ki, :], start=(ki == 0), stop=(ki == 5))
        nc.scalar.copy(kTr[:, oi, :], pq[:])

    # attention per head: scoresT (f32r, padded free) -> exp -> AV (+denom via
    # ones col) -> scale
    eT = pa.tile([P, 2, 2, N], f32, name="eT_xa")
    attn_out = pa.tile([P, 2, 2, C], f32, name="xn_sb")
    rr = pa.tile([P, 2, 2, NH], f32, name="rr")
    for h in range(NH):
        ci, off = h // 2, (h % 2) * HD
        for s2 in range(2):
            # 256-wide window keeps the f32r matmul at 1 cycle/row; for s2=1
            # the window is right-aligned (cols 138:394), so query j lands at
            # psum column 59+j instead of j.
            w0 = 0 if s2 == 0 else T2 - QPAD
            qo = s2 * N - w0
            for (nkt, k0, ksz) in st:
                psc = pp.tile([P, QPAD], f32, space="PSUM", name="ps")
                nc.tensor.matmul(
                    psc[0:ksz, :],
                    lhsT=kTr[off:off + HD, ci, s2 * N + k0: s2 * N + k0 + ksz],
                    rhs=qTr[off:off + HD, ci, w0: w0 + QPAD],
                    start=True, stop=True)
                nc.scalar.activation(eT[0:ksz, s2, nkt, :],
                                     psc[0:ksz, qo: qo + N],
                                     AF.Exp, scale=0.125,
                                     bias=KC["zero"][0:ksz, 0:1])
        for s2 in range(2):
            for (qt, q0, qsz) in st:
                po = pp.tile([P, 65], f32, space="PSUM", name="ps")
                for (nkt, k0, ksz) in st:
                    nc.tensor.matmul(
                        po[0:qsz, :],
                        lhsT=eT[0:ksz, s2, nkt, q0:q0 + qsz],
                        rhs=v_blk[0:ksz, s2, nkt, h, :],
                        start=(nkt == 0), stop=(nkt == 1))
                nc.vector.reciprocal(rr[0:qsz, s2, qt, h:h + 1], po[0:qsz, 0:1])
                nc.vector.tensor_scalar(
                    out=attn_out[0:qsz, s2, qt, h * HD:(h + 1) * HD],
                    in0=po[0:qsz, 1:65],
                    scalar1=rr[0:qsz, s2, qt, h:h + 1],
                    scalar2=None, op0=OP.mult)

    # transpose attn_out -> feature-major f32r (reuses qTr's slot: the score
    # matmuls above are qTr's last readers)
    aoTr = pa.tile([P, 6, T2], f32r, name="qTr")
    for ci in range(6):
        ptr = pp.tile([P, T2], f32, space="PSUM", name="ps")
        for s2 in range(2):
            for (mt, m0, msz) in st:
                nc.tensor.transpose(
                    out=ptr[:, s2 * N + m0: s2 * N + m0 + msz],
                    in_=attn_out[0:msz, s2, mt, ci * P:(ci + 1) * P],
                    identity=ident[0:msz, 0:msz])
        nc.scalar.copy(aoTr[:, ci, :], ptr[:])

    # proj (f32r) + residual -> xattn
    xa_sb = pa.tile([P, 2, 2, C], f32, name="eT_xa")
    for s2 in range(2):
        samp = 2 * ch + s2
        for (mt, m0, msz) in st:
            for (n0, nsz) in ((0, 512), (512, 256)):
                pj = pp.tile([P, 512], f32, space="PSUM", name="ps")
                for ki in range(6):
                    nc.tensor.matmul(
                        pj[0:msz, 0:nsz],
                        lhsT=aoTr[:, ki, s2 * N + m0: s2 * N + m0 + msz],
                        rhs=projr[:, ki, n0:n0 + nsz],
                        start=(ki == 0), stop=(ki == 5))
                nc.vector.tensor_tensor(out=xa_sb[0:msz, s2, mt, n0:n0 + nsz],
                                        in0=x_sb[0:msz, s2, mt, n0:n0 + nsz],
                                        in1=pj[0:msz, 0:nsz], op=OP.add)
            nc.sync.dma_start(out=xattn_d[samp * N + m0: samp * N + m0 + msz, :],
                              in_=xa_sb[0:msz, s2, mt, :])
        # cls row into xsel
        nc.sync.dma_start(out=xsel_d[samp * NO: samp * NO + 1, :],
                          in_=xa_sb[0:1, s2, 0, :])
    return xnT


def _phase_c_chunk(nc, tc, pcs, pp, t0, tsz, xsel_d, fc1_r, fc2_r, ident,
                   out_flat, pc1):
    tiles = _ceil_tiles(tsz)
    nt = len(tiles)
    xc_sb = pcs.tile([P, 4, C], f32, name="xc_sb")
    xn2 = pc1.tile([P, 4, C], f32, name="xn2")
    for ti, (m0, msz) in enumerate(tiles):
        nc.sync.dma_start(out=xc_sb[0:msz, ti, :],
                          in_=xsel_d[t0 + m0: t0 + m0 + msz, :])
        layernorm_tile(nc, pcs, pp, xn2[0:msz, ti, :], xc_sb[0:msz, ti, :],
                       msz, False, "c")
    xnT = pc1.tile([P, 6, 512], bf16, name="xnT2")
    for ci in range(6):
        ptr = pp.tile([P, 512], f32, space="PSUM", name="ps")
        for ti, (m0, msz) in enumerate(tiles):
            nc.tensor.transpose(out=ptr[:, m0:m0 + msz],
                                in_=xn2[0:msz, ti, ci * P:(ci + 1) * P],
                                identity=ident[0:msz, 0:msz])
        nc.vector.tensor_copy(xnT[:, ci, 0:tsz], ptr[:, 0:tsz])

    # fc1 (bf16, resident) + gelu -> hT [H4, tsz] feature-major
    hT = pc1.tile([P, 24, 512], bf16, name="hT")
    for oi in range(24):
        pf = pp.tile([P, 512], f32, space="PSUM", name="ps")
        for ki in range(6):
            nc.tensor.matmul(pf[:, 0:tsz], lhsT=fc1_r[:, ki, oi * P:(oi + 1) * P],
                             rhs=xnT[:, ki, 0:tsz], start=(ki == 0), stop=(ki == 5))
        nc.scalar.activation(hT[:, oi, 0:tsz], pf[:, 0:tsz], AF.Gelu,
                             bias=KC["zero"][:, 0:1])

    # fc2 (bf16) + residual -> out
    for ti, (m0, msz) in enumerate(tiles):
        for (n0, nsz) in ((0, 512), (512, 256)):
            pf = pp.tile([P, 512], f32, space="PSUM", name="ps")
            for ki in range(24):
                nc.tensor.matmul(pf[0:msz, 0:nsz],
                                 lhsT=hT[:, ki, m0:m0 + msz],
                                 rhs=fc2_r[:, ki, n0:n0 + nsz],
                                 start=(ki == 0), stop=(ki == 23))
            nc.vector.tensor_tensor(out=xc_sb[0:msz, ti, n0:n0 + nsz],
                                    in0=xc_sb[0:msz, ti, n0:n0 + nsz],
                                    in1=pf[0:msz, 0:nsz], op=OP.add)
        nc.sync.dma_start(out=out_flat[t0 + m0: t0 + m0 + msz, :],
                          in_=xc_sb[0:msz, ti, :])


_NC_CACHE = None


def kernel(**inputs):
    global _NC_CACHE
    if _NC_CACHE is None:
        _NC_CACHE = build_nc()
    nc = _NC_CACHE

    import ml_dtypes
    x = np.ascontiguousarray(np.asarray(inputs["x"], dtype=np.float32))
    wnames = ["qkv_w", "proj_w", "fc1_w", "fc2_w"]
    ws = {k: np.ascontiguousarray(np.asarray(inputs[k], dtype=np.float32))
          for k in wnames}
    ws["fc1_w"] = ws["fc1_w"].astype(ml_dtypes.bfloat16)
    ws["fc2_w"] = ws["fc2_w"].astype(ml_dtypes.bfloat16)
    B = x.shape[0]
    n_cores = 8
    bl = B // n_cores
    in_maps = []
    for c in range(n_cores):
        m = {"x": x[c * bl:(c + 1) * bl]}
        m.update(ws)
        in_maps.append(m)
    res = run_bass_kernel_spmd(nc, in_maps, core_ids=list(range(n_cores)))
    out = np.concatenate([r["out"] for r in res.results], axis=0)
    return out.astype(np.float32)
